# revision 10
# baseline (speedup 1.0000x reference)
"""ARAP forward kernel for trn2 (8 NeuronCores, SPMD vertex-sharded).

Pair-row gather design (dma_gather has int16 indices, 256B-multiple rows):
  - table [32768, 128] f32 in DRAM, 512B stride.  Pair row r:
      cols [0:24]  raw floats of vertex r      (b-major: x1_b(3), x2_b(3))
      col  24      constant 1.0                (gives sum-w via the PE)
      cols [25:49] raw floats of vertex r+32768
      col  49      constant 1.0
      cols [50:64] zero pad   (cols 64:128 never gathered)
    gather: idx = nbr & 32767 (int16), elem_size = 64 f32 (256B),
    elem_step = 128 (512B stride).  The correct half is selected by the
    weights: w_lo[slot] = w if nbr < 32768 else 0, w_hi mirrors.
  - edge slots: PE-tile j in [0,1024); tile j owns vertices [8j, 8j+8);
    edge (v = 8j+g, k) at partition 16g+k, gather position i = j*128 + p.
    dma_gather places row i at (partition i%128, slot i//128) = (p, j).
  - per chunk (CH=64 tiles): G2 [128, 64, 64]; monomials over both halves
    MONO [128, 64, 2, 72]; per tile 4 fused matmuls with bf16 stationaries
    wsel_lo/wsel_hi [128, 8] (w * blockdiag mask), accumulating in PSUM:
      raw-lo(start) + raw-hi(accum) -> cols [0:25]
      mono-lo(start) + mono-hi(accum) -> cols [25:97]
  - 16 tiles = 1 PSUM fill [128, 97]; ACT-copy to VSUM [128, 64, 97];
    vertex lv = 128*t + p.
  - per-vertex stage on [128, 64, 4] views: assemble S/T/e2 from moments,
    closed-form rotation R = (S^T S)^(-1/2) S^T (Cardano + Franca),
    E_c = (e2_c - 2 (R S^T)_cc + (R T R^T)_cc) * arapWeight.
  - OUT [128, 64, 48] -> DRAM [8192, 48] (per vertex b-major [E(3) | R(9)]).
"""
import numpy as np
import concourse.bass as bass
import concourse.mybir as mybir
import concourse.tile as tile

F32 = mybir.dt.float32
BF16 = mybir.dt.bfloat16
I16 = mybir.dt.int16
AT = mybir.ActivationFunctionType
OP = mybir.AluOpType

B = 4
N = 65536
K = 16
NC = 8
ROWF = 64                 # f32 gathered per row (256B)
TROW = 128                # f32 per table row (512B stride)
RAW = 24                  # raw floats per vertex
SUBW = 25                 # raw + ones col per half
NQ = 18                   # quad monomials per batch
MQ = NQ * B               # 72
VCOL = SUBW + MQ          # 97 psum/vsum cols (24 raw, 1 sumw, 72 mono)
OCOL = 12 * B             # 48 output floats per vertex

NV = N // NC              # 8192
NT = NV // 8              # 1024
EPC = NV * K              # 131072
NIDX = 1024               # indices per dma_gather (HW limit < 2048)
CH = 64                   # tiles per chunk
NCHUNK = NT // CH         # 16
GPI = NIDX // 128         # tiles per gather instr (8)
NFILL = NV // 128         # 64


def set_sizes(n=65536, nidx=1024, chunk_tiles=64):
    global N, NV, NT, EPC, NIDX, CH, NCHUNK, GPI, NFILL, INPUT_SPECS, OUTPUT_SPECS
    N = n
    NV = N // NC
    NT = NV // 8
    EPC = NV * K
    NIDX = nidx
    CH = chunk_tiles
    NCHUNK = NT // CH
    GPI = NIDX // 128
    NFILL = NV // 128
    INPUT_SPECS = {
        "table": ([N // 2, TROW], F32),
        "idx": ([128, EPC // 16], I16),
        "wlo": ([128, NT], F32),
        "whi": ([128, NT], F32),
        "mask": ([128, 128], F32),
        "arap": ([128, 1], F32),
        "xv": ([NV, RAW], F32),
    }
    OUTPUT_SPECS = {"out": ([NV, OCOL], F32)}


# quad monomial table: (src1, c1, src2, c2); src 0 = x1n, 1 = x2n
QUADS = []
for i in range(3):
    for j in range(3):
        QUADS.append((0, i, 1, j))        # q 0..8:  M12[i,j]
SYMQ = {}
for qi, (i, j) in enumerate(((0, 0), (0, 1), (0, 2), (1, 1), (1, 2), (2, 2))):
    QUADS.append((0, i, 0, j))            # q 9..14: M11 sym
    SYMQ[(i, j)] = 9 + qi
for c in range(3):
    QUADS.append((1, c, 1, c))            # q 15..17: M22 diag
assert len(QUADS) == NQ


class Emit:
    """Per-vertex stage emitter with freelist scratch of [128, NFILL, B]."""

    def __init__(self, nc, scratch_pool, tmp_pool, vsum, xv, out_buf, arap,
                 t0=0, t1=None):
        if t1 is None:
            t1 = NFILL
        self.nf = t1 - t0
        self.nc = nc
        self.sp = scratch_pool
        self.tp = tmp_pool
        self.vsum = vsum[:].rearrange("p (t c) -> p t c", c=VCOL)[:, t0:t1, :]
        self.xv4 = xv[:].rearrange("p (t c) -> p t c", c=RAW)[:, t0:t1, :] \
                        .rearrange("p t (b x) -> p t b x", b=B)
        self.outv = out_buf[:].rearrange("p (t c) -> p t c", c=OCOL)[:, t0:t1, :] \
                              .rearrange("p t (b x) -> p t b x", b=B)
        self.arap = arap
        self.free_list = []
        self.n_alloc = 0
        self.n_tt = 0
        self.n_act = 0
        self._bias_tiles = {}

    # ---- views ----
    def Vw(self):
        return self.vsum[:, :, RAW:RAW + 1].to_broadcast([128, self.nf, B])

    def L(self, src, c):
        return self.vsum[:, :, 0:RAW] \
                   .rearrange("p t (b x) -> p t b x", b=B)[:, :, :, src * 3 + c]

    def M(self, q):
        return self.vsum[:, :, SUBW + 4 * q:SUBW + 4 * q + 4]

    def X(self, src, c):
        return self.xv4[:, :, :, src * 3 + c]

    def O(self, col):
        return self.outv[:, :, :, col]

    def S(self, name=None):
        if self.free_list:
            return self.free_list.pop()
        self.n_alloc += 1
        t = self.sp.tile([128, self.nf * B], F32, tag=f"scr{self.n_alloc}")
        return t[:].rearrange("p (t b) -> p t b", b=B)

    def free(self, *aps):
        for ap in aps:
            self.free_list.append(ap)

    def T(self):
        t = self.tp.tile([128, self.nf * B], F32, tag="tmp")
        return t[:].rearrange("p (t b) -> p t b", b=B)

    # ---- ops ----
    def tt(self, out, a, b, op, eng=None):
        (eng or self.nc.vector).tensor_tensor(out=out, in0=a, in1=b, op=op)
        self.n_tt += 1

    def stt(self, out, a, scalar, b, op0, op1, eng=None):
        # scalar_tensor_tensor only lowers on DVE; ignore eng routing
        self.nc.vector.scalar_tensor_tensor(
            out=out, in0=a, scalar=float(scalar), in1=b, op0=op0, op1=op1)
        self.n_tt += 1

    def ts(self, out, a, s1, op0, s2=None, op1=None):
        self.nc.vector.tensor_scalar(
            out=out, in0=a, scalar1=float(s1),
            scalar2=None if s2 is None else float(s2),
            op0=op0, **({} if op1 is None else {"op1": op1}))
        self.n_tt += 1

    def recip(self, out, a):
        self.nc.vector.reciprocal(out=out, in_=a)
        self.n_tt += 1

    def bias_tile(self, val):
        val = float(val)
        if val not in self._bias_tiles:
            t = self.sp.tile([128, 1], F32, tag=f"bias{len(self._bias_tiles)}")
            self.nc.vector.memset(t[:], val)
            self._bias_tiles[val] = t[:]
        return self._bias_tiles[val]

    def act(self, out, a, func, bias=0.0, scale=1.0):
        if func != AT.Copy and float(bias) != 0.0:
            bias = self.bias_tile(bias)
        else:
            bias = float(bias)
        self.nc.scalar.activation(out=out, in_=a, func=func,
                                  bias=bias, scale=float(scale))
        self.n_act += 1


def emit_vertex_stage(em: Emit, use_gp=True):
    nc = em.nc
    gp = nc.gpsimd if use_gp else None
    MUL, ADD, SUB = OP.mult, OP.add, OP.subtract

    # --- assembly of S, T(sym), e2 from moments ---
    u = [em.S() for _ in range(3)]
    v = [em.S() for _ in range(3)]
    y = [em.S() for _ in range(3)]
    for i in range(3):
        em.tt(u[i], em.Vw(), em.X(0, i), MUL)
    for j in range(3):
        em.tt(v[j], em.Vw(), em.X(1, j), MUL, eng=gp)
        em.tt(v[j], v[j], em.L(1, j), SUB, eng=gp)
        em.tt(y[j], u[j], em.L(0, j), SUB)
    Sm = [[em.S() for _ in range(3)] for _ in range(3)]
    for i in range(3):
        for j in range(3):
            t = em.T()
            em.tt(Sm[i][j], em.X(0, i), v[j], MUL)
            em.tt(t, em.L(0, i), em.X(1, j), MUL, eng=gp)
            em.tt(Sm[i][j], Sm[i][j], t, SUB)
            em.tt(Sm[i][j], Sm[i][j], em.M(i * 3 + j), ADD)
    Tm = {}
    for (i, j), q in SYMQ.items():
        t = em.S()
        tt_ = em.T()
        em.tt(t, em.X(0, i), y[j], MUL, eng=gp)
        em.tt(tt_, em.L(0, i), em.X(0, j), MUL)
        em.tt(t, t, tt_, SUB, eng=gp)
        em.tt(t, t, em.M(q), ADD, eng=gp)
        Tm[(i, j)] = t
    e2 = [em.S() for _ in range(3)]
    for c in range(3):
        t = em.T()
        em.tt(t, v[c], em.L(1, c), SUB)
        em.tt(e2[c], em.X(1, c), t, MUL)
        em.tt(e2[c], e2[c], em.M(15 + c), ADD)
    em.free(*u, *v, *y)

    # --- A = S^T S (symmetric) ---
    A = {}
    for i in range(3):
        for j in range(i, 3):
            a = em.S()
            t = em.T()
            em.tt(a, Sm[0][i], Sm[0][j], MUL)
            em.tt(t, Sm[1][i], Sm[1][j], MUL, eng=gp)
            em.tt(a, a, t, ADD)
            t2 = em.T()
            em.tt(t2, Sm[2][i], Sm[2][j], MUL, eng=gp)
            em.tt(a, a, t2, ADD)
            A[(i, j)] = a

    # --- Cardano eigenvalues ---
    tr = em.S()
    em.tt(tr, A[(0, 0)], A[(1, 1)], ADD)
    em.tt(tr, tr, A[(2, 2)], ADD)
    q = em.S()
    em.act(q, tr, AT.Copy, scale=1.0 / 3.0)
    sq12 = em.S()
    em.act(sq12, A[(1, 2)], AT.Square)
    p1 = em.S()
    t = em.T()
    em.act(p1, A[(0, 1)], AT.Square)
    em.act(t, A[(0, 2)], AT.Square)
    em.tt(p1, p1, t, ADD, eng=gp)
    em.tt(p1, p1, sq12, ADD, eng=gp)
    d = [em.S() for _ in range(3)]
    for i in range(3):
        em.tt(d[i], A[(i, i)], q, SUB)
    p2 = em.S()
    t = em.T()
    em.act(p2, d[0], AT.Square)
    em.act(t, d[1], AT.Square)
    em.tt(p2, p2, t, ADD)
    t = em.T()
    em.act(t, d[2], AT.Square)
    em.tt(p2, p2, t, ADD)
    em.stt(p2, p1, 2.0, p2, MUL, ADD)
    p = em.S()
    em.act(p, p2, AT.Sqrt, scale=1.0 / 6.0)
    pinv = em.S()
    em.ts(pinv, p, 1e-30, OP.max)
    em.recip(pinv, pinv)
    em.free(p2)
    t1 = em.S(); t2 = em.S(); t3 = em.S(); detm = em.S()
    em.tt(t1, d[1], d[2], MUL)
    em.tt(t1, t1, sq12, SUB)
    tb = em.T()
    em.tt(t2, A[(0, 1)], d[2], MUL, eng=gp)
    em.tt(tb, A[(1, 2)], A[(0, 2)], MUL, eng=gp)
    em.tt(t2, t2, tb, SUB, eng=gp)
    tb = em.T()
    em.tt(t3, A[(0, 1)], A[(1, 2)], MUL)
    em.tt(tb, d[1], A[(0, 2)], MUL)
    em.tt(t3, t3, tb, SUB)
    tb = em.T()
    em.tt(detm, d[0], t1, MUL)
    em.tt(tb, A[(0, 1)], t2, MUL, eng=gp)
    em.tt(detm, detm, tb, SUB)
    tb = em.T()
    em.tt(tb, A[(0, 2)], t3, MUL)
    em.tt(detm, detm, tb, ADD)
    em.free(t1, t2, t3, *d)
    r = em.S()
    em.tt(r, pinv, pinv, MUL)
    em.tt(r, r, pinv, MUL)
    em.tt(r, r, detm, MUL)
    em.ts(r, r, 0.5, MUL)
    em.ts(r, r, -0.999999, OP.max, 0.999999, OP.min)
    em.free(detm, pinv)
    # x = r / sqrt(1 - r^2); acos(r) = pi/2 - arctan(x).  ACT Arctan only
    # accepts [-pi/2, pi/2], so range-reduce: |x| <= 1 direct, else
    # arctan(|x|) = pi/2 - arctan(1/|x|); restore sign via sign(r).
    rs = em.S()
    em.act(rs, r, AT.Square)
    em.ts(rs, rs, -1.0, OP.mult, 1.0, OP.add)
    em.recip(rs, rs)
    em.act(rs, rs, AT.Sqrt)
    ax = em.S()
    em.tt(ax, rs, r, MUL)
    em.act(ax, ax, AT.Abs)          # |x|
    inv = em.S()
    em.ts(inv, ax, 1e-30, OP.max)
    em.recip(inv, inv)              # 1/|x|
    m = em.S()
    em.tt(m, ax, inv, OP.min)       # min(|x|, 1/|x|) in [0, 1]
    em.act(m, m, AT.Arctan)
    big = em.S()
    em.tt(big, ax, inv, OP.is_gt)   # 1 if |x| > 1/|x|
    # at = m + big*(pi/2 - 2m)
    t = em.T()
    em.ts(t, m, -2.0, OP.mult, float(np.pi / 2), OP.add)
    em.tt(t, t, big, MUL)
    at = m
    em.tt(at, at, t, ADD)
    sg = em.S()
    em.act(sg, r, AT.Sign)
    em.tt(at, at, sg, MUL)          # arctan(x), signed
    # phi = (pi/2 - at)/3 = pi/6 - at/3
    phi = em.S()
    em.act(phi, at, AT.Copy, scale=-1.0 / 3.0, bias=float(np.pi / 6))
    em.free(r, rs, ax, inv, big, sg, at)
    c1 = em.S(); c3 = em.S()
    # cos(phi) = sin(pi/2 - phi); cos(phi + 2pi/3) = sin(-pi/6 - phi)
    em.act(c1, phi, AT.Sin, scale=-1.0, bias=float(np.pi / 2))
    em.act(c3, phi, AT.Sin, scale=-1.0, bias=float(-np.pi / 6))
    em.free(phi)
    l1 = em.S(); l2 = em.S(); l3 = em.S()
    em.tt(l1, p, c1, MUL)
    em.stt(l1, l1, 2.0, q, MUL, ADD)
    em.tt(l3, p, c3, MUL, eng=gp)
    em.stt(l3, l3, 2.0, q, MUL, ADD, eng=gp)
    em.stt(l2, q, 3.0, l1, MUL, SUB)
    em.tt(l2, l2, l3, SUB)
    for l in (l1, l2, l3):
        em.ts(l, l, 0.0, OP.max)
    em.free(p, q, c1, c3)
    # Newton-refine l3 on charpoly of A: the Cardano trig path loses
    # relative precision exactly when l3 << l1 (ACT LUT error ~1e-5 abs
    # in the eigenvalue scale), which wrecks R at near-rank-2 vertices.
    # II = sum of principal 2x2 minors; detA for IIIB too.
    II = em.S()
    t = em.T()
    em.tt(II, A[(0, 0)], A[(1, 1)], MUL)
    em.tt(t, A[(0, 0)], A[(2, 2)], MUL, eng=gp)
    em.tt(II, II, t, ADD)
    t = em.T()
    em.tt(t, A[(1, 1)], A[(2, 2)], MUL, eng=gp)
    em.tt(II, II, t, ADD)
    em.tt(II, II, p1, SUB)
    em.free(p1)
    # detA = det(S)^2: computed from S, not A, so the relative error does
    # not square the condition number (critical for tiny sigma_3 vertices).
    detA = em.S()
    t1d = em.S(); t2d = em.S(); t3d = em.S()
    em.tt(t1d, Sm[1][1], Sm[2][2], MUL)
    t = em.T()
    em.tt(t, Sm[1][2], Sm[2][1], MUL, eng=gp)
    em.tt(t1d, t1d, t, SUB)
    em.tt(t2d, Sm[1][0], Sm[2][2], MUL, eng=gp)
    t = em.T()
    em.tt(t, Sm[1][2], Sm[2][0], MUL)
    em.tt(t2d, t2d, t, SUB, eng=gp)
    em.tt(t3d, Sm[1][0], Sm[2][1], MUL)
    t = em.T()
    em.tt(t, Sm[1][1], Sm[2][0], MUL)
    em.tt(t3d, t3d, t, SUB)
    em.tt(detA, Sm[0][0], t1d, MUL)
    t = em.T()
    em.tt(t, Sm[0][1], t2d, MUL, eng=gp)
    em.tt(detA, detA, t, SUB)
    t = em.T()
    em.tt(t, Sm[0][2], t3d, MUL)
    em.tt(detA, detA, t, ADD)
    em.act(detA, detA, AT.Square)
    em.free(t1d, t2d, t3d, sq12)
    gv_ = em.S(); gd_ = em.S(); lim = em.S(); nlim = em.S()
    # trust region: |step| <= (l2 - l3)/2, so the refinement is inert when
    # l2 ~ l3 (where g' ~ 0 would blow the Newton step, and where the
    # Cardano value is already fine).
    em.tt(lim, l2, l3, SUB)
    em.ts(lim, lim, 0.5, OP.mult, 0.0, OP.max)
    em.ts(nlim, lim, -1.0, OP.mult)
    for _ in range(2):
        # g = ((l3 - tr)*l3 + II)*l3 - detA ; g' = (3*l3 - 2*tr)*l3 + II
        em.tt(gv_, l3, tr, SUB)
        em.tt(gv_, gv_, l3, MUL)
        em.tt(gv_, gv_, II, ADD)
        em.tt(gv_, gv_, l3, MUL)
        em.tt(gv_, gv_, detA, SUB)
        em.ts(gd_, l3, 3.0, OP.mult)
        t = em.T()
        em.stt(t, tr, -2.0, gd_, MUL, ADD)
        em.tt(gd_, t, l3, MUL)
        em.tt(gd_, gd_, II, ADD)
        em.recip(gd_, gd_)
        em.tt(gv_, gv_, gd_, MUL)
        em.tt(gv_, gv_, lim, OP.min)
        em.tt(gv_, gv_, nlim, OP.max)
        em.tt(l3, l3, gv_, SUB)
    em.ts(l3, l3, 0.0, OP.max)
    em.free(gv_, gd_, II, lim, nlim)
    s1 = em.S(); s2 = em.S(); s3 = em.S()
    em.act(s1, l1, AT.Sqrt)
    em.act(s2, l2, AT.Sqrt)
    em.act(s3, l3, AT.Sqrt)
    em.free(l1, l2, l3)
    IB = em.S()
    em.tt(IB, s1, s2, ADD)
    em.tt(IB, IB, s3, ADD)
    IIIB = em.S()
    em.ts(IIIB, detA, 0.0, OP.max)
    em.act(IIIB, IIIB, AT.Sqrt)
    em.free(s1, s2, s3, detA)
    IIB = em.S()
    em.act(IIB, IB, AT.Square)
    em.tt(IIB, IIB, tr, SUB)
    em.ts(IIB, IIB, 0.5, MUL)
    em.free(tr)
    C = {}
    for i in range(3):
        for j in range(i, 3):
            c = em.S()
            em.tt(c, IB, A[(i, j)], MUL, eng=(gp if i != j else None))
            if i == j:
                em.tt(c, c, IIIB, ADD)
            C[(i, j)] = c
    em.free(IB, IIIB)
    adj = {}
    for (i, j), (m1, m2, m3, m4) in {
        (0, 0): ((1, 1), (2, 2), (1, 2), (1, 2)),
        (0, 1): ((0, 2), (1, 2), (0, 1), (2, 2)),
        (0, 2): ((0, 1), (1, 2), (0, 2), (1, 1)),
        (1, 1): ((0, 0), (2, 2), (0, 2), (0, 2)),
        (1, 2): ((0, 1), (0, 2), (0, 0), (1, 2)),
        (2, 2): ((0, 0), (1, 1), (0, 1), (0, 1)),
    }.items():
        a = em.S()
        t = em.T()
        eng = gp if (i + j) % 2 == 1 else None
        em.tt(a, C[m1], C[m2], MUL, eng=eng)
        em.tt(t, C[m3], C[m4], MUL, eng=eng)
        em.tt(a, a, t, SUB, eng=eng)
        adj[(i, j)] = a
    detc = em.S()
    t = em.T()
    em.tt(detc, C[(0, 0)], adj[(0, 0)], MUL)
    em.tt(t, C[(0, 1)], adj[(0, 1)], MUL)
    em.tt(detc, detc, t, ADD)
    t = em.T()
    em.tt(t, C[(0, 2)], adj[(0, 2)], MUL)
    em.tt(detc, detc, t, ADD)
    dinv = em.S()
    em.recip(dinv, detc)
    em.free(detc, *C.values())
    Z = adj
    for (i, j), a in adj.items():
        em.tt(a, a, dinv, MUL, eng=(gp if i != j else None))
    em.free(dinv)

    def sy(Md, i, j):
        return Md[(i, j)] if (i, j) in Md else Md[(j, i)]

    for i in range(3):
        em.tt(A[(i, i)], A[(i, i)], IIB, ADD)
    em.free(IIB)
    Binv = {}
    for i in range(3):
        for j in range(i, 3):
            bb = em.S()
            t = em.T()
            em.tt(bb, sy(A, i, 0), sy(Z, 0, j), MUL)
            em.tt(t, sy(A, i, 1), sy(Z, 1, j), MUL, eng=gp)
            em.tt(bb, bb, t, ADD)
            t = em.T()
            em.tt(t, sy(A, i, 2), sy(Z, 2, j), MUL, eng=gp)
            em.tt(bb, bb, t, ADD)
            Binv[(i, j)] = bb
    em.free(*A.values(), *Z.values())
    R = [[None] * 3 for _ in range(3)]
    for i in range(3):
        for j in range(3):
            rr = em.O(3 + i * 3 + j)
            t = em.T()
            em.tt(rr, sy(Binv, i, 0), Sm[j][0], MUL)
            em.tt(t, sy(Binv, i, 1), Sm[j][1], MUL, eng=gp)
            em.tt(rr, rr, t, ADD)
            t = em.T()
            em.tt(t, sy(Binv, i, 2), Sm[j][2], MUL, eng=gp)
            em.tt(rr, rr, t, ADD)
            R[i][j] = rr
    em.free(*Binv.values())
    for (i, j) in ((0, 1), (0, 2), (1, 2)):
        em.ts(Tm[(i, j)], Tm[(i, j)], 2.0, OP.mult)
    arap_bc = em.arap[:].rearrange("p (x y) -> p x y", x=1) \
                        .to_broadcast([128, em.nf, B])
    for c in range(3):
        mid = em.S()
        t = em.T()
        em.tt(mid, R[c][0], Sm[0][c], MUL)
        em.tt(t, R[c][1], Sm[1][c], MUL, eng=gp)
        em.tt(mid, mid, t, ADD)
        t = em.T()
        em.tt(t, R[c][2], Sm[2][c], MUL)
        em.tt(mid, mid, t, ADD)
        third = em.S()
        em.act(third, R[c][0], AT.Square)
        em.tt(third, third, Tm[(0, 0)], MUL)
        for ii in (1, 2):
            t = em.T()
            em.act(t, R[c][ii], AT.Square)
            em.tt(t, t, Tm[(ii, ii)], MUL, eng=gp)
            em.tt(third, third, t, ADD)
        for (i, j) in ((0, 1), (0, 2), (1, 2)):
            t = em.T()
            em.tt(t, R[c][i], R[c][j], MUL)
            em.tt(t, t, Tm[(i, j)], MUL, eng=gp)
            em.tt(third, third, t, ADD)
        ec = em.O(c)
        em.stt(ec, mid, -2.0, e2[c], MUL, ADD)
        em.tt(ec, ec, third, ADD)
        em.ts(ec, ec, 0.0, OP.max)
        em.tt(ec, ec, arap_bc, MUL)
        em.free(mid, third)


def build_kernel(tc, outs, ins):
    nc = tc.nc
    table = ins["table"]          # [N//2, TROW] f32
    idx = ins["idx"]              # [128, EPC//16] i16 (wrapped, replicated)
    wlo = ins["wlo"]              # [128, NT] f32
    whi = ins["whi"]              # [128, NT] f32
    mask = ins["mask"]            # [128, 8] f32
    arap = ins["arap"]            # [128, 1] f32
    xvsl = ins["xv"]              # [NV, RAW] f32
    out = outs["out"]             # [NV, OCOL] f32

    with (
        tc.tile_pool(name="persist", bufs=1) as pp,
        tc.tile_pool(name="stream", bufs=2) as sp,
        tc.tile_pool(name="single", bufs=1) as sb1,
        tc.tile_pool(name="scratch", bufs=1) as scp,
        tc.tile_pool(name="tmp", bufs=12) as tmp_pool,
        tc.tile_pool(name="psum", bufs=6, space="PSUM") as psp,
    ):
        wlot = pp.tile([128, NT], F32, tag="wlot")
        nc.sync.dma_start(out=wlot[:], in_=wlo[:])
        whit = pp.tile([128, NT], F32, tag="whit")
        nc.sync.dma_start(out=whit[:], in_=whi[:])
        maskt = pp.tile([128, 128], F32, tag="maskt")
        nc.sync.dma_start(out=maskt[:], in_=mask[:])
        arapt = pp.tile([128, 1], F32, tag="arapt")
        nc.sync.dma_start(out=arapt[:], in_=arap[:])
        xv = pp.tile([128, NFILL * RAW], F32, tag="xv")
        nc.sync.dma_start(
            out=xv[:].rearrange("p (t c) -> p t c", c=RAW),
            in_=xvsl[:].rearrange("(t p) c -> p t c", p=128))
        vsum = pp.tile([128, NFILL * VCOL], F32, tag="vsum")
        outb = pp.tile([128, NFILL * OCOL], F32, tag="outb")

        mask4 = maskt[:].rearrange("p (q g) -> p q g", g=32)
        for ch in range(NCHUNK):
            idxc = sp.tile([128, CH * 8], I16, tag="idxc")
            nc.sync.dma_start(out=idxc[:], in_=idx[:, ch * CH * 8:(ch + 1) * CH * 8])
            # wsel32[p, j, g32] = w[p, ch*CH+j] * mask4[p, j%4, g32]
            wsel_lo = sb1.tile([128, CH * 32], F32, tag="wsel_lo")
            wsel_hi = sb1.tile([128, CH * 32], F32, tag="wsel_hi")
            for wsel, wt in ((wsel_lo, wlot), (wsel_hi, whit)):
                nc.vector.tensor_tensor(
                    out=wsel[:].rearrange("p (j q g) -> p j q g", q=4, g=32),
                    in0=wt[:, ch * CH:(ch + 1) * CH]
                        .rearrange("p (j q x) -> p j q x", q=4, x=1)
                        .to_broadcast([128, CH // 4, 4, 32]),
                    in1=mask4.rearrange("p (x q) g -> p x q g", x=1)
                             .to_broadcast([128, CH // 4, 4, 32]),
                    op=OP.mult)
            g2 = sp.tile([128, CH * ROWF], F32, tag="G")
            gv = g2[:].rearrange("p (s c) -> p s c", c=ROWF)
            for gi in range(CH // GPI):
                nc.gpsimd.dma_gather(
                    out_ap=gv[:, gi * GPI:(gi + 1) * GPI, :],
                    in_ap=table[:, 0:ROWF],
                    idxs_ap=idxc[:, gi * GPI * 8:(gi + 1) * GPI * 8],
                    num_idxs=NIDX,
                    num_idxs_reg=NIDX,
                    elem_size=ROWF,
                    elem_step=TROW,
                    queue_num=gi % 4,
                )
            mono = sb1.tile([128, CH * 2 * MQ], F32, tag="MONO")
            monov = mono[:].rearrange("p (s u q) -> p s u q", u=2, q=MQ)
            gvb = gv[:, :, 0:2 * SUBW] \
                .rearrange("p s (u y) -> p s u y", u=2)[:, :, :, 0:RAW] \
                .rearrange("p s u (b x) -> p s u b x", b=B)
            for qi, (s1_, c1_, s2_, c2_) in enumerate(QUADS):
                nc.vector.tensor_tensor(
                    out=monov[:, :, :, qi * B:(qi + 1) * B],
                    in0=gvb[:, :, :, :, s1_ * 3 + c1_],
                    in1=gvb[:, :, :, :, s2_ * 3 + c2_],
                    op=OP.mult)
            wl32 = wsel_lo[:].rearrange("p (s g) -> p s g", g=32)
            wh32 = wsel_hi[:].rearrange("p (s g) -> p s g", g=32)
            for f in range(CH // 16):
                ps = psp.tile([128, VCOL], F32, tag="ps")
                for stq in range(4):           # super-tile within fill
                    base = stq * 32
                    for q in range(4):         # raw group: start..stop
                        sl = f * 16 + stq * 4 + q
                        nc.tensor.matmul(
                            out=ps[base:base + 32, 0:SUBW],
                            lhsT=wl32[:, sl, :],
                            rhs=gv[:, sl, 0:SUBW],
                            start=(q == 0), stop=False,
                            tile_position=(0, base))
                        nc.tensor.matmul(
                            out=ps[base:base + 32, 0:SUBW],
                            lhsT=wh32[:, sl, :],
                            rhs=gv[:, sl, SUBW:2 * SUBW],
                            start=False, stop=(q == 3),
                            tile_position=(0, base))
                    for q in range(4):         # mono group: start..stop
                        sl = f * 16 + stq * 4 + q
                        nc.tensor.matmul(
                            out=ps[base:base + 32, SUBW:VCOL],
                            lhsT=wl32[:, sl, :],
                            rhs=monov[:, sl, 0, :],
                            start=(q == 0), stop=False,
                            tile_position=(0, base))
                        nc.tensor.matmul(
                            out=ps[base:base + 32, SUBW:VCOL],
                            lhsT=wh32[:, sl, :],
                            rhs=monov[:, sl, 1, :],
                            start=False, stop=(q == 3),
                            tile_position=(0, base))
                t = ch * (CH // 16) + f
                nc.scalar.copy(
                    out=vsum[:, t * VCOL:(t + 1) * VCOL], in_=ps[:, :])
            if ch == NCHUNK // 2 - 1:
                em = Emit(nc, scp, tmp_pool, vsum, xv, outb, arapt,
                          0, NFILL // 2)
                emit_vertex_stage(em, use_gp=False)

        em = Emit(nc, scp, tmp_pool, vsum, xv, outb, arapt,
                  NFILL // 2, NFILL)
        emit_vertex_stage(em, use_gp=True)

        nc.sync.dma_start(
            out=out[:].rearrange("(t p) c -> p t c", p=128),
            in_=outb[:].rearrange("p (t c) -> p t c", c=OCOL))


def host_prepare(xyz1, xyz2, neighborList, weightMatrix):
    """Build all per-core input arrays (layout only, no float arithmetic)."""
    raw = np.concatenate([np.asarray(xyz1), np.asarray(xyz2)], axis=2)  # [B,N,6]
    raw = np.transpose(raw, (1, 0, 2)).reshape(N, RAW).astype(np.float32)
    half = N // 2
    table = np.zeros((half, TROW), dtype=np.float32)
    table[:, 0:RAW] = raw[:half]
    table[:, RAW] = 1.0
    table[:, SUBW:SUBW + RAW] = raw[half:]
    table[:, SUBW + RAW] = 1.0

    nbr = np.asarray(neighborList).astype(np.uint32).reshape(N, K)
    wm = np.asarray(weightMatrix).astype(np.float32).reshape(N, K)
    mask32 = np.zeros((128, 4, 32), np.float32)
    for p in range(128):
        for q in range(4):
            mask32[p, q, 8 * q + p // 16] = 1.0
    mask32 = mask32.reshape(128, 128)
    in_maps = []
    for c in range(NC):
        v0 = c * NV
        nbrc = nbr[v0:v0 + NV].reshape(NT, 128)     # [j, p], p = g*16+k
        wc = wm[v0:v0 + NV].reshape(NT, 128)
        is_hi = nbrc >= half
        idx_flat = (nbrc & (half - 1)).astype(np.uint16).reshape(NT * 128)
        wrapped = idx_flat.reshape(-1, 16).T         # [16, EPC/16]
        idxw = np.ascontiguousarray(np.tile(wrapped, (8, 1))).view(np.int16)
        w_lo = np.where(is_hi, 0.0, wc).astype(np.float32).reshape(NT, 128).T
        w_hi = np.where(is_hi, wc, 0.0).astype(np.float32).reshape(NT, 128).T
        in_maps.append({
            "table": table,
            "idx": idxw,
            "wlo": np.ascontiguousarray(w_lo),
            "whi": np.ascontiguousarray(w_hi),
            "mask": mask32,
            "xv": np.ascontiguousarray(raw[v0:v0 + NV]),
        })
    return in_maps


def host_unpack(outs):
    full = np.concatenate(outs, axis=0).reshape(N, B, 12)
    E = np.ascontiguousarray(np.transpose(full[:, :, 0:3], (1, 0, 2)))
    R = np.ascontiguousarray(np.transpose(full[:, :, 3:12], (1, 0, 2)))
    return E, R


INPUT_SPECS = {
    "table": ([N // 2, TROW], F32),
    "idx": ([128, EPC // 16], I16),
    "wlo": ([128, NT], F32),
    "whi": ([128, NT], F32),
    "mask": ([128, 128], F32),
    "arap": ([128, 1], F32),
    "xv": ([NV, RAW], F32),
}
OUTPUT_SPECS = {"out": ([NV, OCOL], F32)}


# ======================================================================
# Self-contained entry point
# ======================================================================
import concourse.bacc as _bacc
from concourse.bass_utils import run_bass_kernel_spmd as _run_spmd
from concourse.bass_interp import get_hw_module as _get_hw_module
from concourse.tile import TileContext as _TileContext

_NC_CACHE = None
LAST_IN_MAPS = None


def _build_nc():
    global _NC_CACHE
    if _NC_CACHE is not None:
        return _NC_CACHE
    nc = _bacc.Bacc("TRN2", target_bir_lowering=False, debug=False,
                    num_devices=NC, num_swdge_queues=4)
    ins_d = {k: nc.dram_tensor(k, list(s), d, kind="ExternalInput").ap()
             for k, (s, d) in INPUT_SPECS.items()}
    outs_d = {k: nc.dram_tensor(k, list(s), d, kind="ExternalOutput").ap()
              for k, (s, d) in OUTPUT_SPECS.items()}
    with _TileContext(nc) as tc:
        build_kernel(tc, outs_d, ins_d)
    nc.compile()
    nc.m = _get_hw_module(nc.m)
    _NC_CACHE = nc
    return nc


def run_on_hw(in_maps, trace=False):
    nc = _build_nc()
    return _run_spmd(nc, in_maps, core_ids=list(range(NC)), trace=trace)


def kernel(xyz1, xyz2, neighborList, numNeighbors, accnumNeighbors,
           weightMatrix, arapWeight):
    global LAST_IN_MAPS
    xyz1 = np.asarray(xyz1, dtype=np.float32)
    xyz2 = np.asarray(xyz2, dtype=np.float32)
    neighborList = np.asarray(neighborList)
    weightMatrix = np.asarray(weightMatrix, dtype=np.float32)
    acc = np.asarray(accnumNeighbors)
    assert np.array_equal(acc, np.arange(N, dtype=acc.dtype) * K), \
        "kernel compiled for uniform CSR (accnum = arange*K)"
    in_maps = host_prepare(xyz1, xyz2, neighborList, weightMatrix)
    arap_val = np.float32(np.asarray(arapWeight).reshape(-1)[0])
    for im in in_maps:
        im["arap"] = np.full((128, 1), arap_val, np.float32)
    LAST_IN_MAPS = in_maps
    res = run_on_hw(in_maps, trace=False)
    outs = [res.results[c]["out"] for c in range(NC)]
    E, R = host_unpack(outs)
    return E.astype(np.float32), R.astype(np.float32)


# revision 11
# speedup vs baseline: 1.2076x; 1.2076x over previous
"""ARAP forward kernel for trn2 (8 NeuronCores, SPMD vertex-sharded).

Pair-row gather design (dma_gather has int16 indices, 256B-multiple rows):
  - table [32768, 128] f32 in DRAM, 512B stride.  Pair row r:
      cols [0:24]  raw floats of vertex r      (b-major: x1_b(3), x2_b(3))
      col  24      constant 1.0                (gives sum-w via the PE)
      cols [25:49] raw floats of vertex r+32768
      col  49      constant 1.0
      cols [50:64] zero pad   (cols 64:128 never gathered)
    gather: idx = nbr & 32767 (int16), elem_size = 64 f32 (256B),
    elem_step = 128 (512B stride).  The correct half is selected by the
    weights: w_lo[slot] = w if nbr < 32768 else 0, w_hi mirrors.
  - edge slots: PE-tile j in [0,1024); tile j owns vertices [8j, 8j+8);
    edge (v = 8j+g, k) at partition 16g+k, gather position i = j*128 + p.
    dma_gather places row i at (partition i%128, slot i//128) = (p, j).
  - per chunk (CH=64 tiles): G2 [128, 64, 64]; monomials over both halves
    MONO [128, 64, 2, 72]; per tile 4 fused matmuls with bf16 stationaries
    wsel_lo/wsel_hi [128, 8] (w * blockdiag mask), accumulating in PSUM:
      raw-lo(start) + raw-hi(accum) -> cols [0:25]
      mono-lo(start) + mono-hi(accum) -> cols [25:97]
  - 16 tiles = 1 PSUM fill [128, 97]; ACT-copy to VSUM [128, 64, 97];
    vertex lv = 128*t + p.
  - per-vertex stage on [128, 64, 4] views: assemble S/T/e2 from moments,
    closed-form rotation R = (S^T S)^(-1/2) S^T (Cardano + Franca),
    E_c = (e2_c - 2 (R S^T)_cc + (R T R^T)_cc) * arapWeight.
  - OUT [128, 64, 48] -> DRAM [8192, 48] (per vertex b-major [E(3) | R(9)]).
"""
import numpy as np
import concourse.bass as bass
import concourse.mybir as mybir
import concourse.tile as tile

F32 = mybir.dt.float32
BF16 = mybir.dt.bfloat16
I16 = mybir.dt.int16
AT = mybir.ActivationFunctionType
OP = mybir.AluOpType

B = 4
N = 65536
K = 16
NC = 8
ROWF = 64                 # f32 gathered per row (256B)
TROW = 128                # f32 per table row (512B stride)
RAW = 24                  # raw floats per vertex
SUBW = 25                 # raw + ones col per half
NQ = 18                   # quad monomials per batch
MQ = NQ * B               # 72
VCOL = SUBW + MQ          # 97 psum/vsum cols (24 raw, 1 sumw, 72 mono)
OCOL = 12 * B             # 48 output floats per vertex

NV = N // NC              # 8192
NT = NV // 8              # 1024
EPC = NV * K              # 131072
NIDX = 1024               # indices per dma_gather (HW limit < 2048)
CH = 64                   # tiles per chunk
NCHUNK = NT // CH         # 16
GPI = NIDX // 128         # tiles per gather instr (8)
NFILL = NV // 128         # 64


def set_sizes(n=65536, nidx=1024, chunk_tiles=64):
    global N, NV, NT, EPC, NIDX, CH, NCHUNK, GPI, NFILL, INPUT_SPECS, OUTPUT_SPECS
    N = n
    NV = N // NC
    NT = NV // 8
    EPC = NV * K
    NIDX = nidx
    CH = chunk_tiles
    NCHUNK = NT // CH
    GPI = NIDX // 128
    NFILL = NV // 128
    INPUT_SPECS = {
        "table": ([N // 2, TROW], F32),
        "idx": ([128, EPC // 16], I16),
        "wlo": ([128, NT], F32),
        "whi": ([128, NT], F32),
        "mask": ([128, 128], F32),
        "arap": ([128, 1], F32),
        "xv": ([NV, RAW], F32),
    }
    OUTPUT_SPECS = {"out": ([NV, OCOL], F32)}


# quad monomial table: (src1, c1, src2, c2); src 0 = x1n, 1 = x2n
QUADS = []
for i in range(3):
    for j in range(3):
        QUADS.append((0, i, 1, j))        # q 0..8:  M12[i,j]
SYMQ = {}
for qi, (i, j) in enumerate(((0, 0), (0, 1), (0, 2), (1, 1), (1, 2), (2, 2))):
    QUADS.append((0, i, 0, j))            # q 9..14: M11 sym
    SYMQ[(i, j)] = 9 + qi
for c in range(3):
    QUADS.append((1, c, 1, c))            # q 15..17: M22 diag
assert len(QUADS) == NQ


class Emit:
    """Per-vertex stage emitter with freelist scratch of [128, NFILL, B]."""

    def __init__(self, nc, scratch_pool, tmp_pool, vsum, xv, out_buf, arap,
                 t0=0, t1=None):
        if t1 is None:
            t1 = NFILL
        self.nf = t1 - t0
        self.nc = nc
        self.sp = scratch_pool
        self.tp = tmp_pool
        self.vsum = vsum[:].rearrange("p (t c) -> p t c", c=VCOL)[:, t0:t1, :]
        self.xv4 = xv[:].rearrange("p (t c) -> p t c", c=RAW)[:, t0:t1, :] \
                        .rearrange("p t (b x) -> p t b x", b=B)
        self.outv = out_buf[:].rearrange("p (t c) -> p t c", c=OCOL)[:, t0:t1, :] \
                              .rearrange("p t (b x) -> p t b x", b=B)
        self.arap = arap
        self.free_list = []
        self.n_alloc = 0
        self.n_tt = 0
        self.n_act = 0
        self._bias_tiles = {}

    # ---- views ----
    def Vw(self):
        return self.vsum[:, :, RAW:RAW + 1].to_broadcast([128, self.nf, B])

    def L(self, src, c):
        return self.vsum[:, :, 0:RAW] \
                   .rearrange("p t (b x) -> p t b x", b=B)[:, :, :, src * 3 + c]

    def M(self, q):
        return self.vsum[:, :, SUBW + 4 * q:SUBW + 4 * q + 4]

    def X(self, src, c):
        return self.xv4[:, :, :, src * 3 + c]

    def O(self, col):
        return self.outv[:, :, :, col]

    def S(self, name=None):
        if self.free_list:
            return self.free_list.pop()
        self.n_alloc += 1
        t = self.sp.tile([128, self.nf * B], F32, tag=f"scr{self.n_alloc}")
        return t[:].rearrange("p (t b) -> p t b", b=B)

    def free(self, *aps):
        for ap in aps:
            self.free_list.append(ap)

    def T(self):
        t = self.tp.tile([128, self.nf * B], F32, tag="tmp")
        return t[:].rearrange("p (t b) -> p t b", b=B)

    # ---- ops ----
    def tt(self, out, a, b, op, eng=None):
        (eng or self.nc.vector).tensor_tensor(out=out, in0=a, in1=b, op=op)
        self.n_tt += 1

    def stt(self, out, a, scalar, b, op0, op1, eng=None):
        # scalar_tensor_tensor only lowers on DVE; ignore eng routing
        self.nc.vector.scalar_tensor_tensor(
            out=out, in0=a, scalar=float(scalar), in1=b, op0=op0, op1=op1)
        self.n_tt += 1

    def ts(self, out, a, s1, op0, s2=None, op1=None):
        self.nc.vector.tensor_scalar(
            out=out, in0=a, scalar1=float(s1),
            scalar2=None if s2 is None else float(s2),
            op0=op0, **({} if op1 is None else {"op1": op1}))
        self.n_tt += 1

    def recip(self, out, a):
        self.nc.vector.reciprocal(out=out, in_=a)
        self.n_tt += 1

    def bias_tile(self, val):
        val = float(val)
        if val not in self._bias_tiles:
            t = self.sp.tile([128, 1], F32, tag=f"bias{len(self._bias_tiles)}")
            self.nc.vector.memset(t[:], val)
            self._bias_tiles[val] = t[:]
        return self._bias_tiles[val]

    def act(self, out, a, func, bias=0.0, scale=1.0):
        if func != AT.Copy and float(bias) != 0.0:
            bias = self.bias_tile(bias)
        else:
            bias = float(bias)
        self.nc.scalar.activation(out=out, in_=a, func=func,
                                  bias=bias, scale=float(scale))
        self.n_act += 1


def emit_vertex_stage(em: Emit, use_gp=True):
    nc = em.nc
    gp = nc.gpsimd if use_gp else None
    MUL, ADD, SUB = OP.mult, OP.add, OP.subtract

    # --- assembly of S, T(sym), e2 from moments ---
    u = [em.S() for _ in range(3)]
    v = [em.S() for _ in range(3)]
    y = [em.S() for _ in range(3)]
    for i in range(3):
        em.tt(u[i], em.Vw(), em.X(0, i), MUL)
    for j in range(3):
        em.tt(v[j], em.Vw(), em.X(1, j), MUL, eng=gp)
        em.tt(v[j], v[j], em.L(1, j), SUB, eng=gp)
        em.tt(y[j], u[j], em.L(0, j), SUB)
    Sm = [[em.S() for _ in range(3)] for _ in range(3)]
    for i in range(3):
        for j in range(3):
            t = em.T()
            em.tt(Sm[i][j], em.X(0, i), v[j], MUL)
            em.tt(t, em.L(0, i), em.X(1, j), MUL, eng=gp)
            em.tt(Sm[i][j], Sm[i][j], t, SUB)
            em.tt(Sm[i][j], Sm[i][j], em.M(i * 3 + j), ADD)
    Tm = {}
    for (i, j), q in SYMQ.items():
        t = em.S()
        tt_ = em.T()
        em.tt(t, em.X(0, i), y[j], MUL, eng=gp)
        em.tt(tt_, em.L(0, i), em.X(0, j), MUL)
        em.tt(t, t, tt_, SUB, eng=gp)
        em.tt(t, t, em.M(q), ADD, eng=gp)
        Tm[(i, j)] = t
    e2 = [em.S() for _ in range(3)]
    for c in range(3):
        t = em.T()
        em.tt(t, v[c], em.L(1, c), SUB)
        em.tt(e2[c], em.X(1, c), t, MUL)
        em.tt(e2[c], e2[c], em.M(15 + c), ADD)
    em.free(*u, *v, *y)

    # --- A = S^T S (symmetric) ---
    A = {}
    for i in range(3):
        for j in range(i, 3):
            a = em.S()
            t = em.T()
            em.tt(a, Sm[0][i], Sm[0][j], MUL)
            em.tt(t, Sm[1][i], Sm[1][j], MUL, eng=gp)
            em.tt(a, a, t, ADD)
            t2 = em.T()
            em.tt(t2, Sm[2][i], Sm[2][j], MUL, eng=gp)
            em.tt(a, a, t2, ADD)
            A[(i, j)] = a

    # --- Cardano eigenvalues ---
    tr = em.S()
    em.tt(tr, A[(0, 0)], A[(1, 1)], ADD)
    em.tt(tr, tr, A[(2, 2)], ADD)
    q = em.S()
    em.act(q, tr, AT.Copy, scale=1.0 / 3.0)
    sq12 = em.S()
    em.act(sq12, A[(1, 2)], AT.Square)
    p1 = em.S()
    t = em.T()
    em.act(p1, A[(0, 1)], AT.Square)
    em.act(t, A[(0, 2)], AT.Square)
    em.tt(p1, p1, t, ADD, eng=gp)
    em.tt(p1, p1, sq12, ADD, eng=gp)
    d = [em.S() for _ in range(3)]
    for i in range(3):
        em.tt(d[i], A[(i, i)], q, SUB)
    p2 = em.S()
    t = em.T()
    em.act(p2, d[0], AT.Square)
    em.act(t, d[1], AT.Square)
    em.tt(p2, p2, t, ADD)
    t = em.T()
    em.act(t, d[2], AT.Square)
    em.tt(p2, p2, t, ADD)
    em.stt(p2, p1, 2.0, p2, MUL, ADD)
    p = em.S()
    em.act(p, p2, AT.Sqrt, scale=1.0 / 6.0)
    pinv = em.S()
    em.ts(pinv, p, 1e-30, OP.max)
    em.recip(pinv, pinv)
    em.free(p2)
    t1 = em.S(); t2 = em.S(); t3 = em.S(); detm = em.S()
    em.tt(t1, d[1], d[2], MUL)
    em.tt(t1, t1, sq12, SUB)
    tb = em.T()
    em.tt(t2, A[(0, 1)], d[2], MUL, eng=gp)
    em.tt(tb, A[(1, 2)], A[(0, 2)], MUL, eng=gp)
    em.tt(t2, t2, tb, SUB, eng=gp)
    tb = em.T()
    em.tt(t3, A[(0, 1)], A[(1, 2)], MUL)
    em.tt(tb, d[1], A[(0, 2)], MUL)
    em.tt(t3, t3, tb, SUB)
    tb = em.T()
    em.tt(detm, d[0], t1, MUL)
    em.tt(tb, A[(0, 1)], t2, MUL, eng=gp)
    em.tt(detm, detm, tb, SUB)
    tb = em.T()
    em.tt(tb, A[(0, 2)], t3, MUL)
    em.tt(detm, detm, tb, ADD)
    em.free(t1, t2, t3, *d)
    r = em.S()
    em.tt(r, pinv, pinv, MUL)
    em.tt(r, r, pinv, MUL)
    em.tt(r, r, detm, MUL)
    em.ts(r, r, 0.5, MUL)
    em.ts(r, r, -0.999999, OP.max, 0.999999, OP.min)
    em.free(detm, pinv)
    # x = r / sqrt(1 - r^2); acos(r) = pi/2 - arctan(x).  ACT Arctan only
    # accepts [-pi/2, pi/2], so range-reduce: |x| <= 1 direct, else
    # arctan(|x|) = pi/2 - arctan(1/|x|); restore sign via sign(r).
    rs = em.S()
    em.act(rs, r, AT.Square)
    em.ts(rs, rs, -1.0, OP.mult, 1.0, OP.add)
    em.recip(rs, rs)
    em.act(rs, rs, AT.Sqrt)
    ax = em.S()
    em.tt(ax, rs, r, MUL)
    em.act(ax, ax, AT.Abs)          # |x|
    inv = em.S()
    em.ts(inv, ax, 1e-30, OP.max)
    em.recip(inv, inv)              # 1/|x|
    m = em.S()
    em.tt(m, ax, inv, OP.min)       # min(|x|, 1/|x|) in [0, 1]
    em.act(m, m, AT.Arctan)
    big = em.S()
    em.tt(big, ax, inv, OP.is_gt)   # 1 if |x| > 1/|x|
    # at = m + big*(pi/2 - 2m)
    t = em.T()
    em.ts(t, m, -2.0, OP.mult, float(np.pi / 2), OP.add)
    em.tt(t, t, big, MUL)
    at = m
    em.tt(at, at, t, ADD)
    sg = em.S()
    em.act(sg, r, AT.Sign)
    em.tt(at, at, sg, MUL)          # arctan(x), signed
    # phi = (pi/2 - at)/3 = pi/6 - at/3
    phi = em.S()
    em.act(phi, at, AT.Copy, scale=-1.0 / 3.0, bias=float(np.pi / 6))
    em.free(r, rs, ax, inv, big, sg, at)
    c1 = em.S(); c3 = em.S()
    # cos(phi) = sin(pi/2 - phi); cos(phi + 2pi/3) = sin(-pi/6 - phi)
    em.act(c1, phi, AT.Sin, scale=-1.0, bias=float(np.pi / 2))
    em.act(c3, phi, AT.Sin, scale=-1.0, bias=float(-np.pi / 6))
    em.free(phi)
    l1 = em.S(); l2 = em.S(); l3 = em.S()
    em.tt(l1, p, c1, MUL)
    em.stt(l1, l1, 2.0, q, MUL, ADD)
    em.tt(l3, p, c3, MUL, eng=gp)
    em.stt(l3, l3, 2.0, q, MUL, ADD, eng=gp)
    em.stt(l2, q, 3.0, l1, MUL, SUB)
    em.tt(l2, l2, l3, SUB)
    for l in (l1, l2, l3):
        em.ts(l, l, 0.0, OP.max)
    em.free(p, q, c1, c3)
    # Newton-refine l3 on charpoly of A: the Cardano trig path loses
    # relative precision exactly when l3 << l1 (ACT LUT error ~1e-5 abs
    # in the eigenvalue scale), which wrecks R at near-rank-2 vertices.
    # II = sum of principal 2x2 minors; detA for IIIB too.
    II = em.S()
    t = em.T()
    em.tt(II, A[(0, 0)], A[(1, 1)], MUL)
    em.tt(t, A[(0, 0)], A[(2, 2)], MUL, eng=gp)
    em.tt(II, II, t, ADD)
    t = em.T()
    em.tt(t, A[(1, 1)], A[(2, 2)], MUL, eng=gp)
    em.tt(II, II, t, ADD)
    em.tt(II, II, p1, SUB)
    em.free(p1)
    # detA = det(S)^2: computed from S, not A, so the relative error does
    # not square the condition number (critical for tiny sigma_3 vertices).
    detA = em.S()
    t1d = em.S(); t2d = em.S(); t3d = em.S()
    em.tt(t1d, Sm[1][1], Sm[2][2], MUL)
    t = em.T()
    em.tt(t, Sm[1][2], Sm[2][1], MUL, eng=gp)
    em.tt(t1d, t1d, t, SUB)
    em.tt(t2d, Sm[1][0], Sm[2][2], MUL, eng=gp)
    t = em.T()
    em.tt(t, Sm[1][2], Sm[2][0], MUL)
    em.tt(t2d, t2d, t, SUB, eng=gp)
    em.tt(t3d, Sm[1][0], Sm[2][1], MUL)
    t = em.T()
    em.tt(t, Sm[1][1], Sm[2][0], MUL)
    em.tt(t3d, t3d, t, SUB)
    em.tt(detA, Sm[0][0], t1d, MUL)
    t = em.T()
    em.tt(t, Sm[0][1], t2d, MUL, eng=gp)
    em.tt(detA, detA, t, SUB)
    t = em.T()
    em.tt(t, Sm[0][2], t3d, MUL)
    em.tt(detA, detA, t, ADD)
    em.act(detA, detA, AT.Square)
    em.free(t1d, t2d, t3d, sq12)
    gv_ = em.S(); gd_ = em.S(); lim = em.S(); nlim = em.S()
    # trust region: |step| <= (l2 - l3)/2, so the refinement is inert when
    # l2 ~ l3 (where g' ~ 0 would blow the Newton step, and where the
    # Cardano value is already fine).
    em.tt(lim, l2, l3, SUB)
    em.ts(lim, lim, 0.5, OP.mult, 0.0, OP.max)
    em.ts(nlim, lim, -1.0, OP.mult)
    for _ in range(2):
        # g = ((l3 - tr)*l3 + II)*l3 - detA ; g' = (3*l3 - 2*tr)*l3 + II
        em.tt(gv_, l3, tr, SUB)
        em.tt(gv_, gv_, l3, MUL)
        em.tt(gv_, gv_, II, ADD)
        em.tt(gv_, gv_, l3, MUL)
        em.tt(gv_, gv_, detA, SUB)
        em.ts(gd_, l3, 3.0, OP.mult)
        t = em.T()
        em.stt(t, tr, -2.0, gd_, MUL, ADD)
        em.tt(gd_, t, l3, MUL)
        em.tt(gd_, gd_, II, ADD)
        em.recip(gd_, gd_)
        em.tt(gv_, gv_, gd_, MUL)
        em.tt(gv_, gv_, lim, OP.min)
        em.tt(gv_, gv_, nlim, OP.max)
        em.tt(l3, l3, gv_, SUB)
    em.ts(l3, l3, 0.0, OP.max)
    em.free(gv_, gd_, II, lim, nlim)
    s1 = em.S(); s2 = em.S(); s3 = em.S()
    em.act(s1, l1, AT.Sqrt)
    em.act(s2, l2, AT.Sqrt)
    em.act(s3, l3, AT.Sqrt)
    em.free(l1, l2, l3)
    IB = em.S()
    em.tt(IB, s1, s2, ADD)
    em.tt(IB, IB, s3, ADD)
    IIIB = em.S()
    em.ts(IIIB, detA, 0.0, OP.max)
    em.act(IIIB, IIIB, AT.Sqrt)
    em.free(s1, s2, s3, detA)
    IIB = em.S()
    em.act(IIB, IB, AT.Square)
    em.tt(IIB, IIB, tr, SUB)
    em.ts(IIB, IIB, 0.5, MUL)
    em.free(tr)
    C = {}
    for i in range(3):
        for j in range(i, 3):
            c = em.S()
            em.tt(c, IB, A[(i, j)], MUL, eng=(gp if i != j else None))
            if i == j:
                em.tt(c, c, IIIB, ADD)
            C[(i, j)] = c
    em.free(IB, IIIB)
    adj = {}
    for (i, j), (m1, m2, m3, m4) in {
        (0, 0): ((1, 1), (2, 2), (1, 2), (1, 2)),
        (0, 1): ((0, 2), (1, 2), (0, 1), (2, 2)),
        (0, 2): ((0, 1), (1, 2), (0, 2), (1, 1)),
        (1, 1): ((0, 0), (2, 2), (0, 2), (0, 2)),
        (1, 2): ((0, 1), (0, 2), (0, 0), (1, 2)),
        (2, 2): ((0, 0), (1, 1), (0, 1), (0, 1)),
    }.items():
        a = em.S()
        t = em.T()
        eng = gp if (i + j) % 2 == 1 else None
        em.tt(a, C[m1], C[m2], MUL, eng=eng)
        em.tt(t, C[m3], C[m4], MUL, eng=eng)
        em.tt(a, a, t, SUB, eng=eng)
        adj[(i, j)] = a
    detc = em.S()
    t = em.T()
    em.tt(detc, C[(0, 0)], adj[(0, 0)], MUL)
    em.tt(t, C[(0, 1)], adj[(0, 1)], MUL)
    em.tt(detc, detc, t, ADD)
    t = em.T()
    em.tt(t, C[(0, 2)], adj[(0, 2)], MUL)
    em.tt(detc, detc, t, ADD)
    dinv = em.S()
    em.recip(dinv, detc)
    em.free(detc, *C.values())
    Z = adj
    for (i, j), a in adj.items():
        em.tt(a, a, dinv, MUL, eng=(gp if i != j else None))
    em.free(dinv)

    def sy(Md, i, j):
        return Md[(i, j)] if (i, j) in Md else Md[(j, i)]

    for i in range(3):
        em.tt(A[(i, i)], A[(i, i)], IIB, ADD)
    em.free(IIB)
    Binv = {}
    for i in range(3):
        for j in range(i, 3):
            bb = em.S()
            t = em.T()
            em.tt(bb, sy(A, i, 0), sy(Z, 0, j), MUL)
            em.tt(t, sy(A, i, 1), sy(Z, 1, j), MUL, eng=gp)
            em.tt(bb, bb, t, ADD)
            t = em.T()
            em.tt(t, sy(A, i, 2), sy(Z, 2, j), MUL, eng=gp)
            em.tt(bb, bb, t, ADD)
            Binv[(i, j)] = bb
    em.free(*A.values(), *Z.values())
    R = [[None] * 3 for _ in range(3)]
    for i in range(3):
        for j in range(3):
            rr = em.O(3 + i * 3 + j)
            t = em.T()
            em.tt(rr, sy(Binv, i, 0), Sm[j][0], MUL)
            em.tt(t, sy(Binv, i, 1), Sm[j][1], MUL, eng=gp)
            em.tt(rr, rr, t, ADD)
            t = em.T()
            em.tt(t, sy(Binv, i, 2), Sm[j][2], MUL, eng=gp)
            em.tt(rr, rr, t, ADD)
            R[i][j] = rr
    em.free(*Binv.values())
    for (i, j) in ((0, 1), (0, 2), (1, 2)):
        em.ts(Tm[(i, j)], Tm[(i, j)], 2.0, OP.mult)
    arap_bc = em.arap[:].rearrange("p (x y) -> p x y", x=1) \
                        .to_broadcast([128, em.nf, B])
    for c in range(3):
        mid = em.S()
        t = em.T()
        em.tt(mid, R[c][0], Sm[0][c], MUL)
        em.tt(t, R[c][1], Sm[1][c], MUL, eng=gp)
        em.tt(mid, mid, t, ADD)
        t = em.T()
        em.tt(t, R[c][2], Sm[2][c], MUL)
        em.tt(mid, mid, t, ADD)
        third = em.S()
        em.act(third, R[c][0], AT.Square)
        em.tt(third, third, Tm[(0, 0)], MUL)
        for ii in (1, 2):
            t = em.T()
            em.act(t, R[c][ii], AT.Square)
            em.tt(t, t, Tm[(ii, ii)], MUL, eng=gp)
            em.tt(third, third, t, ADD)
        for (i, j) in ((0, 1), (0, 2), (1, 2)):
            t = em.T()
            em.tt(t, R[c][i], R[c][j], MUL)
            em.tt(t, t, Tm[(i, j)], MUL, eng=gp)
            em.tt(third, third, t, ADD)
        ec = em.O(c)
        em.stt(ec, mid, -2.0, e2[c], MUL, ADD)
        em.tt(ec, ec, third, ADD)
        em.ts(ec, ec, 0.0, OP.max)
        em.tt(ec, ec, arap_bc, MUL)
        em.free(mid, third)


def build_kernel(tc, outs, ins):
    nc = tc.nc
    table = ins["table"]          # [N//2, TROW] f32
    idx = ins["idx"]              # [128, EPC//16] i16 (wrapped, replicated)
    wlo = ins["wlo"]              # [128, NT] f32
    whi = ins["whi"]              # [128, NT] f32
    mask = ins["mask"]            # [128, 8] f32
    arap = ins["arap"]            # [128, 1] f32
    xvsl = ins["xv"]              # [NV, RAW] f32
    out = outs["out"]             # [NV, OCOL] f32

    with (
        tc.tile_pool(name="persist", bufs=1) as pp,
        tc.tile_pool(name="stream", bufs=2) as sp,
        tc.tile_pool(name="single", bufs=1) as sb1,
        tc.tile_pool(name="scratch", bufs=1) as scp,
        tc.tile_pool(name="tmp", bufs=12) as tmp_pool,
        tc.tile_pool(name="psum", bufs=6, space="PSUM") as psp,
    ):
        wlot = pp.tile([128, NT], F32, tag="wlot")
        nc.sync.dma_start(out=wlot[:], in_=wlo[:])
        whit = pp.tile([128, NT], F32, tag="whit")
        nc.sync.dma_start(out=whit[:], in_=whi[:])
        maskt = pp.tile([128, 128], F32, tag="maskt")
        nc.sync.dma_start(out=maskt[:], in_=mask[:])
        arapt = pp.tile([128, 1], F32, tag="arapt")
        nc.sync.dma_start(out=arapt[:], in_=arap[:])
        xv = pp.tile([128, NFILL * RAW], F32, tag="xv")
        nc.sync.dma_start(
            out=xv[:].rearrange("p (t c) -> p t c", c=RAW),
            in_=xvsl[:].rearrange("(t p) c -> p t c", p=128))
        vsum = pp.tile([128, NFILL * VCOL], F32, tag="vsum")
        outb = pp.tile([128, NFILL * OCOL], F32, tag="outb")

        mask4 = maskt[:].rearrange("p (q g) -> p q g", g=32)
        for ch in range(NCHUNK):
            idxc = sp.tile([128, CH * 8], I16, tag="idxc")
            nc.sync.dma_start(out=idxc[:], in_=idx[:, ch * CH * 8:(ch + 1) * CH * 8])
            # wsel32[p, j, g32] = w[p, ch*CH+j] * mask4[p, j%4, g32]
            wsel_lo = sb1.tile([128, CH * 32], F32, tag="wsel_lo")
            wsel_hi = sb1.tile([128, CH * 32], F32, tag="wsel_hi")
            for wsel, wt in ((wsel_lo, wlot), (wsel_hi, whit)):
                nc.vector.tensor_tensor(
                    out=wsel[:].rearrange("p (j q g) -> p j q g", q=4, g=32),
                    in0=wt[:, ch * CH:(ch + 1) * CH]
                        .rearrange("p (j q x) -> p j q x", q=4, x=1)
                        .to_broadcast([128, CH // 4, 4, 32]),
                    in1=mask4.rearrange("p (x q) g -> p x q g", x=1)
                             .to_broadcast([128, CH // 4, 4, 32]),
                    op=OP.mult)
            g2 = sp.tile([128, CH * ROWF], F32, tag="G")
            gv = g2[:].rearrange("p (s c) -> p s c", c=ROWF)
            for gi in range(CH // GPI):
                nc.gpsimd.dma_gather(
                    out_ap=gv[:, gi * GPI:(gi + 1) * GPI, :],
                    in_ap=table[:, 0:ROWF],
                    idxs_ap=idxc[:, gi * GPI * 8:(gi + 1) * GPI * 8],
                    num_idxs=NIDX,
                    num_idxs_reg=NIDX,
                    elem_size=ROWF,
                    elem_step=TROW,
                    queue_num=gi % 4,
                )
            mono = sb1.tile([128, CH * 2 * MQ], F32, tag="MONO")
            monov = mono[:].rearrange("p (s u q) -> p s u q", u=2, q=MQ)
            gvb = gv[:, :, 0:2 * SUBW] \
                .rearrange("p s (u y) -> p s u y", u=2)[:, :, :, 0:RAW] \
                .rearrange("p s u (b x) -> p s u b x", b=B)
            for qi, (s1_, c1_, s2_, c2_) in enumerate(QUADS):
                nc.vector.tensor_tensor(
                    out=monov[:, :, :, qi * B:(qi + 1) * B],
                    in0=gvb[:, :, :, :, s1_ * 3 + c1_],
                    in1=gvb[:, :, :, :, s2_ * 3 + c2_],
                    op=OP.mult)
            wl32 = wsel_lo[:].rearrange("p (s g) -> p s g", g=32)
            wh32 = wsel_hi[:].rearrange("p (s g) -> p s g", g=32)
            for f in range(CH // 16):
                ps = psp.tile([128, VCOL], F32, tag="ps")
                for stq in range(4):           # super-tile within fill
                    base = stq * 32
                    for q in range(4):         # raw group: start..stop
                        sl = f * 16 + stq * 4 + q
                        nc.tensor.matmul(
                            out=ps[base:base + 32, 0:SUBW],
                            lhsT=wl32[:, sl, :],
                            rhs=gv[:, sl, 0:SUBW],
                            start=(q == 0), stop=False,
                            tile_position=(0, base))
                        nc.tensor.matmul(
                            out=ps[base:base + 32, 0:SUBW],
                            lhsT=wh32[:, sl, :],
                            rhs=gv[:, sl, SUBW:2 * SUBW],
                            start=False, stop=(q == 3),
                            tile_position=(0, base))
                    for q in range(4):         # mono group: start..stop
                        sl = f * 16 + stq * 4 + q
                        nc.tensor.matmul(
                            out=ps[base:base + 32, SUBW:VCOL],
                            lhsT=wl32[:, sl, :],
                            rhs=monov[:, sl, 0, :],
                            start=(q == 0), stop=False,
                            tile_position=(0, base))
                        nc.tensor.matmul(
                            out=ps[base:base + 32, SUBW:VCOL],
                            lhsT=wh32[:, sl, :],
                            rhs=monov[:, sl, 1, :],
                            start=False, stop=(q == 3),
                            tile_position=(0, base))
                t = ch * (CH // 16) + f
                nc.scalar.copy(
                    out=vsum[:, t * VCOL:(t + 1) * VCOL], in_=ps[:, :])
            if ch == NCHUNK // 2 - 1:
                em = Emit(nc, scp, tmp_pool, vsum, xv, outb, arapt,
                          0, NFILL // 2)
                emit_vertex_stage(em, use_gp=False)

        em = Emit(nc, scp, tmp_pool, vsum, xv, outb, arapt,
                  NFILL // 2, NFILL)
        emit_vertex_stage(em, use_gp=False)

        nc.sync.dma_start(
            out=out[:].rearrange("(t p) c -> p t c", p=128),
            in_=outb[:].rearrange("p (t c) -> p t c", c=OCOL))


def host_prepare(xyz1, xyz2, neighborList, weightMatrix):
    """Build all per-core input arrays (layout only, no float arithmetic)."""
    raw = np.concatenate([np.asarray(xyz1), np.asarray(xyz2)], axis=2)  # [B,N,6]
    raw = np.transpose(raw, (1, 0, 2)).reshape(N, RAW).astype(np.float32)
    half = N // 2
    table = np.zeros((half, TROW), dtype=np.float32)
    table[:, 0:RAW] = raw[:half]
    table[:, RAW] = 1.0
    table[:, SUBW:SUBW + RAW] = raw[half:]
    table[:, SUBW + RAW] = 1.0

    nbr = np.asarray(neighborList).astype(np.uint32).reshape(N, K)
    wm = np.asarray(weightMatrix).astype(np.float32).reshape(N, K)
    mask32 = np.zeros((128, 4, 32), np.float32)
    for p in range(128):
        for q in range(4):
            mask32[p, q, 8 * q + p // 16] = 1.0
    mask32 = mask32.reshape(128, 128)
    in_maps = []
    for c in range(NC):
        v0 = c * NV
        nbrc = nbr[v0:v0 + NV].reshape(NT, 128)     # [j, p], p = g*16+k
        wc = wm[v0:v0 + NV].reshape(NT, 128)
        is_hi = nbrc >= half
        idx_flat = (nbrc & (half - 1)).astype(np.uint16).reshape(NT * 128)
        wrapped = idx_flat.reshape(-1, 16).T         # [16, EPC/16]
        idxw = np.ascontiguousarray(np.tile(wrapped, (8, 1))).view(np.int16)
        w_lo = np.where(is_hi, 0.0, wc).astype(np.float32).reshape(NT, 128).T
        w_hi = np.where(is_hi, wc, 0.0).astype(np.float32).reshape(NT, 128).T
        in_maps.append({
            "table": table,
            "idx": idxw,
            "wlo": np.ascontiguousarray(w_lo),
            "whi": np.ascontiguousarray(w_hi),
            "mask": mask32,
            "xv": np.ascontiguousarray(raw[v0:v0 + NV]),
        })
    return in_maps


def host_unpack(outs):
    full = np.concatenate(outs, axis=0).reshape(N, B, 12)
    E = np.ascontiguousarray(np.transpose(full[:, :, 0:3], (1, 0, 2)))
    R = np.ascontiguousarray(np.transpose(full[:, :, 3:12], (1, 0, 2)))
    return E, R


INPUT_SPECS = {
    "table": ([N // 2, TROW], F32),
    "idx": ([128, EPC // 16], I16),
    "wlo": ([128, NT], F32),
    "whi": ([128, NT], F32),
    "mask": ([128, 128], F32),
    "arap": ([128, 1], F32),
    "xv": ([NV, RAW], F32),
}
OUTPUT_SPECS = {"out": ([NV, OCOL], F32)}


# ======================================================================
# Self-contained entry point
# ======================================================================
import concourse.bacc as _bacc
from concourse.bass_utils import run_bass_kernel_spmd as _run_spmd
from concourse.bass_interp import get_hw_module as _get_hw_module
from concourse.tile import TileContext as _TileContext

_NC_CACHE = None
LAST_IN_MAPS = None


def _build_nc():
    global _NC_CACHE
    if _NC_CACHE is not None:
        return _NC_CACHE
    nc = _bacc.Bacc("TRN2", target_bir_lowering=False, debug=False,
                    num_devices=NC, num_swdge_queues=4)
    ins_d = {k: nc.dram_tensor(k, list(s), d, kind="ExternalInput").ap()
             for k, (s, d) in INPUT_SPECS.items()}
    outs_d = {k: nc.dram_tensor(k, list(s), d, kind="ExternalOutput").ap()
              for k, (s, d) in OUTPUT_SPECS.items()}
    with _TileContext(nc) as tc:
        build_kernel(tc, outs_d, ins_d)
    nc.compile()
    nc.m = _get_hw_module(nc.m)
    _NC_CACHE = nc
    return nc


def run_on_hw(in_maps, trace=False):
    nc = _build_nc()
    return _run_spmd(nc, in_maps, core_ids=list(range(NC)), trace=trace)


def kernel(xyz1, xyz2, neighborList, numNeighbors, accnumNeighbors,
           weightMatrix, arapWeight):
    global LAST_IN_MAPS
    xyz1 = np.asarray(xyz1, dtype=np.float32)
    xyz2 = np.asarray(xyz2, dtype=np.float32)
    neighborList = np.asarray(neighborList)
    weightMatrix = np.asarray(weightMatrix, dtype=np.float32)
    acc = np.asarray(accnumNeighbors)
    assert np.array_equal(acc, np.arange(N, dtype=acc.dtype) * K), \
        "kernel compiled for uniform CSR (accnum = arange*K)"
    in_maps = host_prepare(xyz1, xyz2, neighborList, weightMatrix)
    arap_val = np.float32(np.asarray(arapWeight).reshape(-1)[0])
    for im in in_maps:
        im["arap"] = np.full((128, 1), arap_val, np.float32)
    LAST_IN_MAPS = in_maps
    res = run_on_hw(in_maps, trace=False)
    outs = [res.results[c]["out"] for c in range(NC)]
    E, R = host_unpack(outs)
    return E.astype(np.float32), R.astype(np.float32)


# revision 12
# speedup vs baseline: 1.2079x; 1.0002x over previous
"""ARAP forward kernel for trn2 (8 NeuronCores, SPMD vertex-sharded).

Pair-row gather design (dma_gather has int16 indices, 256B-multiple rows):
  - table [32768, 128] f32 in DRAM, 512B stride.  Pair row r:
      cols [0:24]  raw floats of vertex r      (b-major: x1_b(3), x2_b(3))
      col  24      constant 1.0                (gives sum-w via the PE)
      cols [25:49] raw floats of vertex r+32768
      col  49      constant 1.0
      cols [50:64] zero pad   (cols 64:128 never gathered)
    gather: idx = nbr & 32767 (int16), elem_size = 64 f32 (256B),
    elem_step = 128 (512B stride).  The correct half is selected by the
    weights: w_lo[slot] = w if nbr < 32768 else 0, w_hi mirrors.
  - edge slots: PE-tile j in [0,1024); tile j owns vertices [8j, 8j+8);
    edge (v = 8j+g, k) at partition 16g+k, gather position i = j*128 + p.
    dma_gather places row i at (partition i%128, slot i//128) = (p, j).
  - per chunk (CH=64 tiles): G2 [128, 64, 64]; monomials over both halves
    MONO [128, 64, 2, 72]; per tile 4 fused matmuls with bf16 stationaries
    wsel_lo/wsel_hi [128, 8] (w * blockdiag mask), accumulating in PSUM:
      raw-lo(start) + raw-hi(accum) -> cols [0:25]
      mono-lo(start) + mono-hi(accum) -> cols [25:97]
  - 16 tiles = 1 PSUM fill [128, 97]; ACT-copy to VSUM [128, 64, 97];
    vertex lv = 128*t + p.
  - per-vertex stage on [128, 64, 4] views: assemble S/T/e2 from moments,
    closed-form rotation R = (S^T S)^(-1/2) S^T (Cardano + Franca),
    E_c = (e2_c - 2 (R S^T)_cc + (R T R^T)_cc) * arapWeight.
  - OUT [128, 64, 48] -> DRAM [8192, 48] (per vertex b-major [E(3) | R(9)]).
"""
import numpy as np
import concourse.bass as bass
import concourse.mybir as mybir
import concourse.tile as tile

F32 = mybir.dt.float32
BF16 = mybir.dt.bfloat16
I16 = mybir.dt.int16
AT = mybir.ActivationFunctionType
OP = mybir.AluOpType

B = 4
N = 65536
K = 16
NC = 8
ROWF = 64                 # f32 gathered per row (256B)
TROW = 128                # f32 per table row (512B stride)
RAW = 24                  # raw floats per vertex
SUBW = 25                 # raw + ones col per half
NQ = 18                   # quad monomials per batch
MQ = NQ * B               # 72
VCOL = SUBW + MQ          # 97 psum/vsum cols (24 raw, 1 sumw, 72 mono)
OCOL = 12 * B             # 48 output floats per vertex

NV = N // NC              # 8192
NT = NV // 8              # 1024
EPC = NV * K              # 131072
NIDX = 1024               # indices per dma_gather (HW limit < 2048)
CH = 64                   # tiles per chunk
NCHUNK = NT // CH         # 16
GPI = NIDX // 128         # tiles per gather instr (8)
NFILL = NV // 128         # 64


def set_sizes(n=65536, nidx=1024, chunk_tiles=64):
    global N, NV, NT, EPC, NIDX, CH, NCHUNK, GPI, NFILL, INPUT_SPECS, OUTPUT_SPECS
    N = n
    NV = N // NC
    NT = NV // 8
    EPC = NV * K
    NIDX = nidx
    CH = chunk_tiles
    NCHUNK = NT // CH
    GPI = NIDX // 128
    NFILL = NV // 128
    INPUT_SPECS = {
        "table": ([N // 2, TROW], F32),
        "idx": ([128, EPC // 16], I16),
        "wlo": ([128, NT], F32),
        "whi": ([128, NT], F32),
        "mask": ([128, 128], F32),
        "arap": ([128, 1], F32),
        "xv": ([NV, RAW], F32),
    }
    OUTPUT_SPECS = {"out": ([NV, OCOL], F32)}


# quad monomial table: (src1, c1, src2, c2); src 0 = x1n, 1 = x2n
QUADS = []
for i in range(3):
    for j in range(3):
        QUADS.append((0, i, 1, j))        # q 0..8:  M12[i,j]
SYMQ = {}
for qi, (i, j) in enumerate(((0, 0), (0, 1), (0, 2), (1, 1), (1, 2), (2, 2))):
    QUADS.append((0, i, 0, j))            # q 9..14: M11 sym
    SYMQ[(i, j)] = 9 + qi
for c in range(3):
    QUADS.append((1, c, 1, c))            # q 15..17: M22 diag
assert len(QUADS) == NQ


class Emit:
    """Per-vertex stage emitter with freelist scratch of [128, NFILL, B]."""

    def __init__(self, nc, scratch_pool, tmp_pool, vsum, xv, out_buf, arap,
                 t0=0, t1=None):
        if t1 is None:
            t1 = NFILL
        self.nf = t1 - t0
        self.nc = nc
        self.sp = scratch_pool
        self.tp = tmp_pool
        self.vsum = vsum[:].rearrange("p (t c) -> p t c", c=VCOL)[:, t0:t1, :]
        self.xv4 = xv[:].rearrange("p (t c) -> p t c", c=RAW)[:, t0:t1, :] \
                        .rearrange("p t (b x) -> p t b x", b=B)
        self.outv = out_buf[:].rearrange("p (t c) -> p t c", c=OCOL)[:, t0:t1, :] \
                              .rearrange("p t (b x) -> p t b x", b=B)
        self.arap = arap
        self.free_list = []
        self.n_alloc = 0
        self.n_tt = 0
        self.n_act = 0
        self._bias_tiles = {}

    # ---- views ----
    def Vw(self):
        return self.vsum[:, :, RAW:RAW + 1].to_broadcast([128, self.nf, B])

    def L(self, src, c):
        return self.vsum[:, :, 0:RAW] \
                   .rearrange("p t (b x) -> p t b x", b=B)[:, :, :, src * 3 + c]

    def M(self, q):
        return self.vsum[:, :, SUBW + 4 * q:SUBW + 4 * q + 4]

    def X(self, src, c):
        return self.xv4[:, :, :, src * 3 + c]

    def O(self, col):
        return self.outv[:, :, :, col]

    def S(self, name=None):
        if self.free_list:
            return self.free_list.pop()
        self.n_alloc += 1
        t = self.sp.tile([128, self.nf * B], F32, tag=f"scr{self.n_alloc}")
        return t[:].rearrange("p (t b) -> p t b", b=B)

    def free(self, *aps):
        for ap in aps:
            self.free_list.append(ap)

    def T(self):
        t = self.tp.tile([128, self.nf * B], F32, tag="tmp")
        return t[:].rearrange("p (t b) -> p t b", b=B)

    # ---- ops ----
    def tt(self, out, a, b, op, eng=None):
        (eng or self.nc.vector).tensor_tensor(out=out, in0=a, in1=b, op=op)
        self.n_tt += 1

    def stt(self, out, a, scalar, b, op0, op1, eng=None):
        # scalar_tensor_tensor only lowers on DVE; ignore eng routing
        self.nc.vector.scalar_tensor_tensor(
            out=out, in0=a, scalar=float(scalar), in1=b, op0=op0, op1=op1)
        self.n_tt += 1

    def ts(self, out, a, s1, op0, s2=None, op1=None):
        self.nc.vector.tensor_scalar(
            out=out, in0=a, scalar1=float(s1),
            scalar2=None if s2 is None else float(s2),
            op0=op0, **({} if op1 is None else {"op1": op1}))
        self.n_tt += 1

    def recip(self, out, a):
        self.nc.vector.reciprocal(out=out, in_=a)
        self.n_tt += 1

    def bias_tile(self, val):
        val = float(val)
        if val not in self._bias_tiles:
            t = self.sp.tile([128, 1], F32, tag=f"bias{len(self._bias_tiles)}")
            self.nc.vector.memset(t[:], val)
            self._bias_tiles[val] = t[:]
        return self._bias_tiles[val]

    def act(self, out, a, func, bias=0.0, scale=1.0):
        if func != AT.Copy and float(bias) != 0.0:
            bias = self.bias_tile(bias)
        else:
            bias = float(bias)
        self.nc.scalar.activation(out=out, in_=a, func=func,
                                  bias=bias, scale=float(scale))
        self.n_act += 1


def emit_vertex_stage(em: Emit, use_gp=True):
    nc = em.nc
    gp = nc.gpsimd if use_gp else None
    MUL, ADD, SUB = OP.mult, OP.add, OP.subtract

    # --- assembly of S, T(sym), e2 from moments ---
    u = [em.S() for _ in range(3)]
    v = [em.S() for _ in range(3)]
    y = [em.S() for _ in range(3)]
    for i in range(3):
        em.tt(u[i], em.Vw(), em.X(0, i), MUL)
    for j in range(3):
        em.tt(v[j], em.Vw(), em.X(1, j), MUL, eng=gp)
        em.tt(v[j], v[j], em.L(1, j), SUB, eng=gp)
        em.tt(y[j], u[j], em.L(0, j), SUB)
    Sm = [[em.S() for _ in range(3)] for _ in range(3)]
    for i in range(3):
        for j in range(3):
            t = em.T()
            em.tt(Sm[i][j], em.X(0, i), v[j], MUL)
            em.tt(t, em.L(0, i), em.X(1, j), MUL, eng=gp)
            em.tt(Sm[i][j], Sm[i][j], t, SUB)
            em.tt(Sm[i][j], Sm[i][j], em.M(i * 3 + j), ADD)
    Tm = {}
    for (i, j), q in SYMQ.items():
        t = em.S()
        tt_ = em.T()
        em.tt(t, em.X(0, i), y[j], MUL, eng=gp)
        em.tt(tt_, em.L(0, i), em.X(0, j), MUL)
        em.tt(t, t, tt_, SUB, eng=gp)
        em.tt(t, t, em.M(q), ADD, eng=gp)
        Tm[(i, j)] = t
    e2 = [em.S() for _ in range(3)]
    for c in range(3):
        t = em.T()
        em.tt(t, v[c], em.L(1, c), SUB)
        em.tt(e2[c], em.X(1, c), t, MUL)
        em.tt(e2[c], e2[c], em.M(15 + c), ADD)
    em.free(*u, *v, *y)

    # --- A = S^T S (symmetric) ---
    A = {}
    for i in range(3):
        for j in range(i, 3):
            a = em.S()
            t = em.T()
            em.tt(a, Sm[0][i], Sm[0][j], MUL)
            em.tt(t, Sm[1][i], Sm[1][j], MUL, eng=gp)
            em.tt(a, a, t, ADD)
            t2 = em.T()
            em.tt(t2, Sm[2][i], Sm[2][j], MUL, eng=gp)
            em.tt(a, a, t2, ADD)
            A[(i, j)] = a

    # --- Cardano eigenvalues ---
    tr = em.S()
    em.tt(tr, A[(0, 0)], A[(1, 1)], ADD)
    em.tt(tr, tr, A[(2, 2)], ADD)
    q = em.S()
    em.act(q, tr, AT.Copy, scale=1.0 / 3.0)
    sq12 = em.S()
    em.act(sq12, A[(1, 2)], AT.Square)
    p1 = em.S()
    t = em.T()
    em.act(p1, A[(0, 1)], AT.Square)
    em.act(t, A[(0, 2)], AT.Square)
    em.tt(p1, p1, t, ADD, eng=gp)
    em.tt(p1, p1, sq12, ADD, eng=gp)
    d = [em.S() for _ in range(3)]
    for i in range(3):
        em.tt(d[i], A[(i, i)], q, SUB)
    p2 = em.S()
    t = em.T()
    em.act(p2, d[0], AT.Square)
    em.act(t, d[1], AT.Square)
    em.tt(p2, p2, t, ADD)
    t = em.T()
    em.act(t, d[2], AT.Square)
    em.tt(p2, p2, t, ADD)
    em.stt(p2, p1, 2.0, p2, MUL, ADD)
    p = em.S()
    em.act(p, p2, AT.Sqrt, scale=1.0 / 6.0)
    pinv = em.S()
    em.ts(pinv, p, 1e-30, OP.max)
    em.recip(pinv, pinv)
    em.free(p2)
    t1 = em.S(); t2 = em.S(); t3 = em.S(); detm = em.S()
    em.tt(t1, d[1], d[2], MUL)
    em.tt(t1, t1, sq12, SUB)
    tb = em.T()
    em.tt(t2, A[(0, 1)], d[2], MUL, eng=gp)
    em.tt(tb, A[(1, 2)], A[(0, 2)], MUL, eng=gp)
    em.tt(t2, t2, tb, SUB, eng=gp)
    tb = em.T()
    em.tt(t3, A[(0, 1)], A[(1, 2)], MUL)
    em.tt(tb, d[1], A[(0, 2)], MUL)
    em.tt(t3, t3, tb, SUB)
    tb = em.T()
    em.tt(detm, d[0], t1, MUL)
    em.tt(tb, A[(0, 1)], t2, MUL, eng=gp)
    em.tt(detm, detm, tb, SUB)
    tb = em.T()
    em.tt(tb, A[(0, 2)], t3, MUL)
    em.tt(detm, detm, tb, ADD)
    em.free(t1, t2, t3, *d)
    r = em.S()
    em.tt(r, pinv, pinv, MUL)
    em.tt(r, r, pinv, MUL)
    em.tt(r, r, detm, MUL)
    em.ts(r, r, 0.5, MUL)
    em.ts(r, r, -0.999999, OP.max, 0.999999, OP.min)
    em.free(detm, pinv)
    # x = r / sqrt(1 - r^2); acos(r) = pi/2 - arctan(x).  ACT Arctan only
    # accepts [-pi/2, pi/2], so range-reduce: |x| <= 1 direct, else
    # arctan(|x|) = pi/2 - arctan(1/|x|); restore sign via sign(r).
    rs = em.S()
    em.act(rs, r, AT.Square)
    em.ts(rs, rs, -1.0, OP.mult, 1.0, OP.add)
    em.recip(rs, rs)
    em.act(rs, rs, AT.Sqrt)
    ax = em.S()
    em.tt(ax, rs, r, MUL)
    em.act(ax, ax, AT.Abs)          # |x|
    inv = em.S()
    em.ts(inv, ax, 1e-30, OP.max)
    em.recip(inv, inv)              # 1/|x|
    m = em.S()
    em.tt(m, ax, inv, OP.min)       # min(|x|, 1/|x|) in [0, 1]
    em.act(m, m, AT.Arctan)
    big = em.S()
    em.tt(big, ax, inv, OP.is_gt)   # 1 if |x| > 1/|x|
    # at = m + big*(pi/2 - 2m)
    t = em.T()
    em.ts(t, m, -2.0, OP.mult, float(np.pi / 2), OP.add)
    em.tt(t, t, big, MUL)
    at = m
    em.tt(at, at, t, ADD)
    sg = em.S()
    em.act(sg, r, AT.Sign)
    em.tt(at, at, sg, MUL)          # arctan(x), signed
    # phi = (pi/2 - at)/3 = pi/6 - at/3
    phi = em.S()
    em.act(phi, at, AT.Copy, scale=-1.0 / 3.0, bias=float(np.pi / 6))
    em.free(r, rs, ax, inv, big, sg, at)
    c1 = em.S(); c3 = em.S()
    # cos(phi) = sin(pi/2 - phi); cos(phi + 2pi/3) = sin(-pi/6 - phi)
    em.act(c1, phi, AT.Sin, scale=-1.0, bias=float(np.pi / 2))
    em.act(c3, phi, AT.Sin, scale=-1.0, bias=float(-np.pi / 6))
    em.free(phi)
    l1 = em.S(); l2 = em.S(); l3 = em.S()
    em.tt(l1, p, c1, MUL)
    em.stt(l1, l1, 2.0, q, MUL, ADD)
    em.tt(l3, p, c3, MUL, eng=gp)
    em.stt(l3, l3, 2.0, q, MUL, ADD, eng=gp)
    em.stt(l2, q, 3.0, l1, MUL, SUB)
    em.tt(l2, l2, l3, SUB)
    for l in (l1, l2, l3):
        em.ts(l, l, 0.0, OP.max)
    em.free(p, q, c1, c3)
    # Newton-refine l3 on charpoly of A: the Cardano trig path loses
    # relative precision exactly when l3 << l1 (ACT LUT error ~1e-5 abs
    # in the eigenvalue scale), which wrecks R at near-rank-2 vertices.
    # II = sum of principal 2x2 minors; detA for IIIB too.
    II = em.S()
    t = em.T()
    em.tt(II, A[(0, 0)], A[(1, 1)], MUL)
    em.tt(t, A[(0, 0)], A[(2, 2)], MUL, eng=gp)
    em.tt(II, II, t, ADD)
    t = em.T()
    em.tt(t, A[(1, 1)], A[(2, 2)], MUL, eng=gp)
    em.tt(II, II, t, ADD)
    em.tt(II, II, p1, SUB)
    em.free(p1)
    # detA = det(S)^2: computed from S, not A, so the relative error does
    # not square the condition number (critical for tiny sigma_3 vertices).
    detA = em.S()
    t1d = em.S(); t2d = em.S(); t3d = em.S()
    em.tt(t1d, Sm[1][1], Sm[2][2], MUL)
    t = em.T()
    em.tt(t, Sm[1][2], Sm[2][1], MUL, eng=gp)
    em.tt(t1d, t1d, t, SUB)
    em.tt(t2d, Sm[1][0], Sm[2][2], MUL, eng=gp)
    t = em.T()
    em.tt(t, Sm[1][2], Sm[2][0], MUL)
    em.tt(t2d, t2d, t, SUB, eng=gp)
    em.tt(t3d, Sm[1][0], Sm[2][1], MUL)
    t = em.T()
    em.tt(t, Sm[1][1], Sm[2][0], MUL)
    em.tt(t3d, t3d, t, SUB)
    em.tt(detA, Sm[0][0], t1d, MUL)
    t = em.T()
    em.tt(t, Sm[0][1], t2d, MUL, eng=gp)
    em.tt(detA, detA, t, SUB)
    t = em.T()
    em.tt(t, Sm[0][2], t3d, MUL)
    em.tt(detA, detA, t, ADD)
    em.act(detA, detA, AT.Square)
    em.free(t1d, t2d, t3d, sq12)
    gv_ = em.S(); gd_ = em.S(); lim = em.S(); nlim = em.S()
    # trust region: |step| <= (l2 - l3)/2, so the refinement is inert when
    # l2 ~ l3 (where g' ~ 0 would blow the Newton step, and where the
    # Cardano value is already fine).
    em.tt(lim, l2, l3, SUB)
    em.ts(lim, lim, 0.5, OP.mult, 0.0, OP.max)
    em.ts(nlim, lim, -1.0, OP.mult)
    for _ in range(2):
        # g = ((l3 - tr)*l3 + II)*l3 - detA ; g' = (3*l3 - 2*tr)*l3 + II
        em.tt(gv_, l3, tr, SUB)
        em.tt(gv_, gv_, l3, MUL)
        em.tt(gv_, gv_, II, ADD)
        em.tt(gv_, gv_, l3, MUL)
        em.tt(gv_, gv_, detA, SUB)
        em.ts(gd_, l3, 3.0, OP.mult)
        t = em.T()
        em.stt(t, tr, -2.0, gd_, MUL, ADD)
        em.tt(gd_, t, l3, MUL)
        em.tt(gd_, gd_, II, ADD)
        em.recip(gd_, gd_)
        em.tt(gv_, gv_, gd_, MUL)
        em.tt(gv_, gv_, lim, OP.min)
        em.tt(gv_, gv_, nlim, OP.max)
        em.tt(l3, l3, gv_, SUB)
    em.ts(l3, l3, 0.0, OP.max)
    em.free(gv_, gd_, II, lim, nlim)
    s1 = em.S(); s2 = em.S(); s3 = em.S()
    em.act(s1, l1, AT.Sqrt)
    em.act(s2, l2, AT.Sqrt)
    em.act(s3, l3, AT.Sqrt)
    em.free(l1, l2, l3)
    IB = em.S()
    em.tt(IB, s1, s2, ADD)
    em.tt(IB, IB, s3, ADD)
    IIIB = em.S()
    em.ts(IIIB, detA, 0.0, OP.max)
    em.act(IIIB, IIIB, AT.Sqrt)
    em.free(s1, s2, s3, detA)
    IIB = em.S()
    em.act(IIB, IB, AT.Square)
    em.tt(IIB, IIB, tr, SUB)
    em.ts(IIB, IIB, 0.5, MUL)
    em.free(tr)
    C = {}
    for i in range(3):
        for j in range(i, 3):
            c = em.S()
            em.tt(c, IB, A[(i, j)], MUL, eng=(gp if i != j else None))
            if i == j:
                em.tt(c, c, IIIB, ADD)
            C[(i, j)] = c
    em.free(IB, IIIB)
    adj = {}
    for (i, j), (m1, m2, m3, m4) in {
        (0, 0): ((1, 1), (2, 2), (1, 2), (1, 2)),
        (0, 1): ((0, 2), (1, 2), (0, 1), (2, 2)),
        (0, 2): ((0, 1), (1, 2), (0, 2), (1, 1)),
        (1, 1): ((0, 0), (2, 2), (0, 2), (0, 2)),
        (1, 2): ((0, 1), (0, 2), (0, 0), (1, 2)),
        (2, 2): ((0, 0), (1, 1), (0, 1), (0, 1)),
    }.items():
        a = em.S()
        t = em.T()
        eng = gp if (i + j) % 2 == 1 else None
        em.tt(a, C[m1], C[m2], MUL, eng=eng)
        em.tt(t, C[m3], C[m4], MUL, eng=eng)
        em.tt(a, a, t, SUB, eng=eng)
        adj[(i, j)] = a
    detc = em.S()
    t = em.T()
    em.tt(detc, C[(0, 0)], adj[(0, 0)], MUL)
    em.tt(t, C[(0, 1)], adj[(0, 1)], MUL)
    em.tt(detc, detc, t, ADD)
    t = em.T()
    em.tt(t, C[(0, 2)], adj[(0, 2)], MUL)
    em.tt(detc, detc, t, ADD)
    dinv = em.S()
    em.recip(dinv, detc)
    em.free(detc, *C.values())
    Z = adj
    for (i, j), a in adj.items():
        em.tt(a, a, dinv, MUL, eng=(gp if i != j else None))
    em.free(dinv)

    def sy(Md, i, j):
        return Md[(i, j)] if (i, j) in Md else Md[(j, i)]

    for i in range(3):
        em.tt(A[(i, i)], A[(i, i)], IIB, ADD)
    em.free(IIB)
    Binv = {}
    for i in range(3):
        for j in range(i, 3):
            bb = em.S()
            t = em.T()
            em.tt(bb, sy(A, i, 0), sy(Z, 0, j), MUL)
            em.tt(t, sy(A, i, 1), sy(Z, 1, j), MUL, eng=gp)
            em.tt(bb, bb, t, ADD)
            t = em.T()
            em.tt(t, sy(A, i, 2), sy(Z, 2, j), MUL, eng=gp)
            em.tt(bb, bb, t, ADD)
            Binv[(i, j)] = bb
    em.free(*A.values(), *Z.values())
    R = [[None] * 3 for _ in range(3)]
    for i in range(3):
        for j in range(3):
            rr = em.O(3 + i * 3 + j)
            t = em.T()
            em.tt(rr, sy(Binv, i, 0), Sm[j][0], MUL)
            em.tt(t, sy(Binv, i, 1), Sm[j][1], MUL, eng=gp)
            em.tt(rr, rr, t, ADD)
            t = em.T()
            em.tt(t, sy(Binv, i, 2), Sm[j][2], MUL, eng=gp)
            em.tt(rr, rr, t, ADD)
            R[i][j] = rr
    em.free(*Binv.values())
    for (i, j) in ((0, 1), (0, 2), (1, 2)):
        em.ts(Tm[(i, j)], Tm[(i, j)], 2.0, OP.mult)
    arap_bc = em.arap[:].rearrange("p (x y) -> p x y", x=1) \
                        .to_broadcast([128, em.nf, B])
    for c in range(3):
        mid = em.S()
        t = em.T()
        em.tt(mid, R[c][0], Sm[0][c], MUL)
        em.tt(t, R[c][1], Sm[1][c], MUL, eng=gp)
        em.tt(mid, mid, t, ADD)
        t = em.T()
        em.tt(t, R[c][2], Sm[2][c], MUL)
        em.tt(mid, mid, t, ADD)
        third = em.S()
        em.act(third, R[c][0], AT.Square)
        em.tt(third, third, Tm[(0, 0)], MUL)
        for ii in (1, 2):
            t = em.T()
            em.act(t, R[c][ii], AT.Square)
            em.tt(t, t, Tm[(ii, ii)], MUL, eng=gp)
            em.tt(third, third, t, ADD)
        for (i, j) in ((0, 1), (0, 2), (1, 2)):
            t = em.T()
            em.tt(t, R[c][i], R[c][j], MUL)
            em.tt(t, t, Tm[(i, j)], MUL, eng=gp)
            em.tt(third, third, t, ADD)
        ec = em.O(c)
        em.stt(ec, mid, -2.0, e2[c], MUL, ADD)
        em.tt(ec, ec, third, ADD)
        em.ts(ec, ec, 0.0, OP.max)
        em.tt(ec, ec, arap_bc, MUL)
        em.free(mid, third)


def build_kernel(tc, outs, ins):
    nc = tc.nc
    table = ins["table"]          # [N//2, TROW] f32
    idx = ins["idx"]              # [128, EPC//16] i16 (wrapped, replicated)
    wlo = ins["wlo"]              # [128, NT] f32
    whi = ins["whi"]              # [128, NT] f32
    mask = ins["mask"]            # [128, 8] f32
    arap = ins["arap"]            # [128, 1] f32
    xvsl = ins["xv"]              # [NV, RAW] f32
    out = outs["out"]             # [NV, OCOL] f32

    with (
        tc.tile_pool(name="persist", bufs=1) as pp,
        tc.tile_pool(name="stream", bufs=2) as sp,
        tc.tile_pool(name="single", bufs=1) as sb1,
        tc.tile_pool(name="scratch", bufs=1) as scp,
        tc.tile_pool(name="tmp", bufs=12) as tmp_pool,
        tc.tile_pool(name="psum", bufs=6, space="PSUM") as psp,
    ):
        wlot = pp.tile([128, NT], F32, tag="wlot")
        nc.sync.dma_start(out=wlot[:], in_=wlo[:])
        whit = pp.tile([128, NT], F32, tag="whit")
        nc.sync.dma_start(out=whit[:], in_=whi[:])
        maskt = pp.tile([128, 128], F32, tag="maskt")
        nc.sync.dma_start(out=maskt[:], in_=mask[:])
        arapt = pp.tile([128, 1], F32, tag="arapt")
        nc.sync.dma_start(out=arapt[:], in_=arap[:])
        xv = pp.tile([128, NFILL * RAW], F32, tag="xv")
        nc.sync.dma_start(
            out=xv[:].rearrange("p (t c) -> p t c", c=RAW),
            in_=xvsl[:].rearrange("(t p) c -> p t c", p=128))
        vsum = pp.tile([128, NFILL * VCOL], F32, tag="vsum")
        outb = pp.tile([128, NFILL * OCOL], F32, tag="outb")

        mask4 = maskt[:].rearrange("p (q g) -> p q g", g=32)
        for ch in range(NCHUNK):
            idxc = sp.tile([128, CH * 8], I16, tag="idxc")
            nc.sync.dma_start(out=idxc[:], in_=idx[:, ch * CH * 8:(ch + 1) * CH * 8])
            # wsel32[p, j, g32] = w[p, ch*CH+j] * mask4[p, j%4, g32]
            wsel_lo = sb1.tile([128, CH * 32], F32, tag="wsel_lo")
            wsel_hi = sb1.tile([128, CH * 32], F32, tag="wsel_hi")
            for wsel, wt in ((wsel_lo, wlot), (wsel_hi, whit)):
                nc.vector.tensor_tensor(
                    out=wsel[:].rearrange("p (j q g) -> p j q g", q=4, g=32),
                    in0=wt[:, ch * CH:(ch + 1) * CH]
                        .rearrange("p (j q x) -> p j q x", q=4, x=1)
                        .to_broadcast([128, CH // 4, 4, 32]),
                    in1=mask4.rearrange("p (x q) g -> p x q g", x=1)
                             .to_broadcast([128, CH // 4, 4, 32]),
                    op=OP.mult)
            g2 = sp.tile([128, CH * ROWF], F32, tag="G")
            gv = g2[:].rearrange("p (s c) -> p s c", c=ROWF)
            for gi in range(CH // GPI):
                nc.gpsimd.dma_gather(
                    out_ap=gv[:, gi * GPI:(gi + 1) * GPI, :],
                    in_ap=table[:, 0:ROWF],
                    idxs_ap=idxc[:, gi * GPI * 8:(gi + 1) * GPI * 8],
                    num_idxs=NIDX,
                    num_idxs_reg=NIDX,
                    elem_size=ROWF,
                    elem_step=TROW,
                    queue_num=gi % 4,
                )
            mono = sb1.tile([128, CH * 2 * MQ], F32, tag="MONO")
            monov = mono[:].rearrange("p (s u q) -> p s u q", u=2, q=MQ)
            gvb = gv[:, :, 0:2 * SUBW] \
                .rearrange("p s (u y) -> p s u y", u=2)[:, :, :, 0:RAW] \
                .rearrange("p s u (b x) -> p s u b x", b=B)
            for qi, (s1_, c1_, s2_, c2_) in enumerate(QUADS):
                nc.vector.tensor_tensor(
                    out=monov[:, :, :, qi * B:(qi + 1) * B],
                    in0=gvb[:, :, :, :, s1_ * 3 + c1_],
                    in1=gvb[:, :, :, :, s2_ * 3 + c2_],
                    op=OP.mult)
            wl32 = wsel_lo[:].rearrange("p (s g) -> p s g", g=32)
            wh32 = wsel_hi[:].rearrange("p (s g) -> p s g", g=32)
            for f in range(CH // 16):
                ps = psp.tile([128, VCOL], F32, tag="ps")
                for stq in range(4):           # super-tile within fill
                    base = stq * 32
                    for q in range(4):         # raw group: start..stop
                        sl = f * 16 + stq * 4 + q
                        nc.tensor.matmul(
                            out=ps[base:base + 32, 0:SUBW],
                            lhsT=wl32[:, sl, :],
                            rhs=gv[:, sl, 0:SUBW],
                            start=(q == 0), stop=False,
                            tile_position=(0, base))
                        nc.tensor.matmul(
                            out=ps[base:base + 32, 0:SUBW],
                            lhsT=wh32[:, sl, :],
                            rhs=gv[:, sl, SUBW:2 * SUBW],
                            start=False, stop=(q == 3),
                            tile_position=(0, base))
                    for q in range(4):         # mono group: start..stop
                        sl = f * 16 + stq * 4 + q
                        nc.tensor.matmul(
                            out=ps[base:base + 32, SUBW:VCOL],
                            lhsT=wl32[:, sl, :],
                            rhs=monov[:, sl, 0, :],
                            start=(q == 0), stop=False,
                            tile_position=(0, base))
                        nc.tensor.matmul(
                            out=ps[base:base + 32, SUBW:VCOL],
                            lhsT=wh32[:, sl, :],
                            rhs=monov[:, sl, 1, :],
                            start=False, stop=(q == 3),
                            tile_position=(0, base))
                t = ch * (CH // 16) + f
                nc.scalar.copy(
                    out=vsum[:, t * VCOL:(t + 1) * VCOL], in_=ps[:, :])
            if ch == (3 * NCHUNK) // 4 - 1:
                em = Emit(nc, scp, tmp_pool, vsum, xv, outb, arapt,
                          0, (3 * NFILL) // 4)
                emit_vertex_stage(em, use_gp=False)

        em = Emit(nc, scp, tmp_pool, vsum, xv, outb, arapt,
                  (3 * NFILL) // 4, NFILL)
        emit_vertex_stage(em, use_gp=False)

        nc.sync.dma_start(
            out=out[:].rearrange("(t p) c -> p t c", p=128),
            in_=outb[:].rearrange("p (t c) -> p t c", c=OCOL))


def host_prepare(xyz1, xyz2, neighborList, weightMatrix):
    """Build all per-core input arrays (layout only, no float arithmetic)."""
    raw = np.concatenate([np.asarray(xyz1), np.asarray(xyz2)], axis=2)  # [B,N,6]
    raw = np.transpose(raw, (1, 0, 2)).reshape(N, RAW).astype(np.float32)
    half = N // 2
    table = np.zeros((half, TROW), dtype=np.float32)
    table[:, 0:RAW] = raw[:half]
    table[:, RAW] = 1.0
    table[:, SUBW:SUBW + RAW] = raw[half:]
    table[:, SUBW + RAW] = 1.0

    nbr = np.asarray(neighborList).astype(np.uint32).reshape(N, K)
    wm = np.asarray(weightMatrix).astype(np.float32).reshape(N, K)
    mask32 = np.zeros((128, 4, 32), np.float32)
    for p in range(128):
        for q in range(4):
            mask32[p, q, 8 * q + p // 16] = 1.0
    mask32 = mask32.reshape(128, 128)
    in_maps = []
    for c in range(NC):
        v0 = c * NV
        nbrc = nbr[v0:v0 + NV].reshape(NT, 128)     # [j, p], p = g*16+k
        wc = wm[v0:v0 + NV].reshape(NT, 128)
        is_hi = nbrc >= half
        idx_flat = (nbrc & (half - 1)).astype(np.uint16).reshape(NT * 128)
        wrapped = idx_flat.reshape(-1, 16).T         # [16, EPC/16]
        idxw = np.ascontiguousarray(np.tile(wrapped, (8, 1))).view(np.int16)
        w_lo = np.where(is_hi, 0.0, wc).astype(np.float32).reshape(NT, 128).T
        w_hi = np.where(is_hi, wc, 0.0).astype(np.float32).reshape(NT, 128).T
        in_maps.append({
            "table": table,
            "idx": idxw,
            "wlo": np.ascontiguousarray(w_lo),
            "whi": np.ascontiguousarray(w_hi),
            "mask": mask32,
            "xv": np.ascontiguousarray(raw[v0:v0 + NV]),
        })
    return in_maps


def host_unpack(outs):
    full = np.concatenate(outs, axis=0).reshape(N, B, 12)
    E = np.ascontiguousarray(np.transpose(full[:, :, 0:3], (1, 0, 2)))
    R = np.ascontiguousarray(np.transpose(full[:, :, 3:12], (1, 0, 2)))
    return E, R


INPUT_SPECS = {
    "table": ([N // 2, TROW], F32),
    "idx": ([128, EPC // 16], I16),
    "wlo": ([128, NT], F32),
    "whi": ([128, NT], F32),
    "mask": ([128, 128], F32),
    "arap": ([128, 1], F32),
    "xv": ([NV, RAW], F32),
}
OUTPUT_SPECS = {"out": ([NV, OCOL], F32)}


# ======================================================================
# Self-contained entry point
# ======================================================================
import concourse.bacc as _bacc
from concourse.bass_utils import run_bass_kernel_spmd as _run_spmd
from concourse.bass_interp import get_hw_module as _get_hw_module
from concourse.tile import TileContext as _TileContext

_NC_CACHE = None
LAST_IN_MAPS = None


def _build_nc():
    global _NC_CACHE
    if _NC_CACHE is not None:
        return _NC_CACHE
    nc = _bacc.Bacc("TRN2", target_bir_lowering=False, debug=False,
                    num_devices=NC, num_swdge_queues=4)
    ins_d = {k: nc.dram_tensor(k, list(s), d, kind="ExternalInput").ap()
             for k, (s, d) in INPUT_SPECS.items()}
    outs_d = {k: nc.dram_tensor(k, list(s), d, kind="ExternalOutput").ap()
              for k, (s, d) in OUTPUT_SPECS.items()}
    with _TileContext(nc) as tc:
        build_kernel(tc, outs_d, ins_d)
    nc.compile()
    nc.m = _get_hw_module(nc.m)
    _NC_CACHE = nc
    return nc


def run_on_hw(in_maps, trace=False):
    nc = _build_nc()
    return _run_spmd(nc, in_maps, core_ids=list(range(NC)), trace=trace)


def kernel(xyz1, xyz2, neighborList, numNeighbors, accnumNeighbors,
           weightMatrix, arapWeight):
    global LAST_IN_MAPS
    xyz1 = np.asarray(xyz1, dtype=np.float32)
    xyz2 = np.asarray(xyz2, dtype=np.float32)
    neighborList = np.asarray(neighborList)
    weightMatrix = np.asarray(weightMatrix, dtype=np.float32)
    acc = np.asarray(accnumNeighbors)
    assert np.array_equal(acc, np.arange(N, dtype=acc.dtype) * K), \
        "kernel compiled for uniform CSR (accnum = arange*K)"
    in_maps = host_prepare(xyz1, xyz2, neighborList, weightMatrix)
    arap_val = np.float32(np.asarray(arapWeight).reshape(-1)[0])
    for im in in_maps:
        im["arap"] = np.full((128, 1), arap_val, np.float32)
    LAST_IN_MAPS = in_maps
    res = run_on_hw(in_maps, trace=False)
    outs = [res.results[c]["out"] for c in range(NC)]
    E, R = host_unpack(outs)
    return E.astype(np.float32), R.astype(np.float32)
